# revision 13
# baseline (speedup 1.0000x reference)
"""Trainium2 Bass kernel for nn_AttentionCircuit (moe_routing).

Math (per batch b):
  P_qk = x_b @ qk_neurons.T            [S, NPOOL]   (dense "router" matmul)
  act[s,n] = P_qk[s, ci_qk[s,n]]
  Q = sum_n (act*gQ)[s,n] * qk_neurons[ci_qk[s,n]]  (ditto K with gK, V w/ v pool)
  causal MHA (H=16, dh=64) + W_O

Key identity: with G[s,p] = sum_{n: ci[s,n]=p} g[s,n] (host-built scatter of
the gates) the gathered reconstruction collapses to dense algebra:
  Q = (P ⊙ G_Q) @ N        =>   Q^T = N^T @ (P^T ⊙ G_Q^T)
so the MoE routing becomes two dense matmuls + one elementwise gate, with
P^T = N @ x^T computed directly in pool-major layout (no device transposes,
no gather/scatter instructions; duplicate indices handled by the host sum).

Sharding: 8 cores = (batch b = c//2) x (sequence half h = c%2). Each core:
  - routes its own 512 tokens (P^T, R^T = P^T ⊙ G^T, Q^T/K^T/V recon)
  - AllGathers K^T and V across its pair (same batch)
  - computes causal attention for its 512 queries over all 16 heads
    (causality via per-core host-built {0,1} masks applied to exp'd scores;
     softmax denominator comes free from a [V_h | ones] stationary operand)
  - applies W_O and writes out^T [D, 512] for its tokens.

dtypes: routing matmuls bf16 (fp32 PSUM accumulation); attention value path
and W_O in fp32r (full matmul rate, ~1.6e-4 error measured on HW).
"""

import os
import numpy as np
import ml_dtypes

import concourse.mybir as mybir
import concourse.tile as tile
from concourse import bacc
from concourse.bass_utils import run_bass_kernel_spmd

B, S, D = 4, 1024, 1024
H = 16
K_SEL = 32
N_POOL = 4096
N_CORES = 8
TOK = 512           # tokens per core
DH = D // H         # 64
PC = N_POOL // 128  # 32 pool chunks
DC = D // 128       # 8 feature chunks
TT = TOK // 128     # 4 token tiles
ST = S // 128       # 8 key tiles

BF16 = mybir.dt.bfloat16
F32 = mybir.dt.float32
F32R = mybir.dt.float32r

REPLICA_GROUPS = [[0, 1], [2, 3], [4, 5], [6, 7]]

_CACHE = {}


def _route(nc, p_st, p_r, ps_pool, xt_sb, ntb_dram, gate_specs, ntb_dt=BF16):
    """P^T = N @ x^T per 128-row pool tile, then R^T = P^T ⊙ G^T.
    gate_specs: list of (gate_dram, gate_dt, r_dt, prefix)."""
    r_tiles = [[] for _ in gate_specs]
    for m in range(PC):
        ntb = p_st.tile([128, D], ntb_dt, name=f"ntb_{ntb_dram.name}_{m}",
                        tag=f"ntb_{ntb_dram.name}", bufs=4)
        nc.sync.dma_start(out=ntb[:], in_=ntb_dram[m])
        pt = ps_pool.tile([128, TOK], F32, name=f"pt_{ntb_dram.name}_{m}",
                          tag="pt", bufs=3)
        for kc in range(DC):
            nc.tensor.matmul(
                pt[:],
                ntb[:, kc * 128:(kc + 1) * 128],
                xt_sb[kc][:],
                start=(kc == 0),
                stop=(kc == DC - 1),
            )
        for gi, (g_dram, g_dt, r_dt, pref) in enumerate(gate_specs):
            g = p_st.tile([128, TOK], g_dt, name=f"g_{pref}_{m}",
                          tag=f"g{pref}", bufs=3)
            nc.sync.dma_start(out=g[:], in_=g_dram[m * 128:(m + 1) * 128, :])
            r = p_r.tile([128, TOK], r_dt, name=f"r_{pref}_{m}",
                         tag=f"r{pref}{m}")
            nc.vector.tensor_mul(r[:], pt[:], g[:])
            r_tiles[gi].append(r)
    return r_tiles


def _recon_T(nc, p_st, p_qkt, ps_pool, n_dram, r_sb, out_dt, n_dt, pref):
    """out^T[d, tok] accumulated over pool chunks -> 8 sbuf tiles [128, TOK]."""
    acc = [ps_pool.tile([128, TOK], F32, name=f"acc_{pref}_{dt}",
                        tag=f"acc{dt}") for dt in range(DC)]
    for pc in range(PC):
        nchunk = p_st.tile([128, D], n_dt, name=f"nch_{pref}_{pc}",
                           tag="nchunk", bufs=3)
        nc.sync.dma_start(out=nchunk[:], in_=n_dram[pc * 128:(pc + 1) * 128, :])
        for dt in range(DC):
            nc.tensor.matmul(
                acc[dt][:],
                nchunk[:, dt * 128:(dt + 1) * 128],
                r_sb[pc][:],
                start=(pc == 0),
                stop=(pc == PC - 1),
            )
    outs = []
    for dt in range(DC):
        o = p_qkt.tile([128, TOK], out_dt, name=f"{pref}_{dt}", tag=f"{pref}{dt}")
        nc.scalar.copy(o[:], acc[dt][:])
        outs.append(o)
    return outs


def _build_nc():
    nc = bacc.Bacc("TRN2", target_bir_lowering=False, debug=False,
                   num_devices=N_CORES)

    # ---- per-core external inputs -------------------------------------
    XT = nc.dram_tensor("XT", [D, TOK], BF16, kind="ExternalInput")
    XTF = nc.dram_tensor("XTF", [D, TOK], F32R, kind="ExternalInput")
    NTQKB = nc.dram_tensor("NTQKB", [PC, 128, D], BF16, kind="ExternalInput")
    NTVB = nc.dram_tensor("NTVB", [PC, 128, D], F32R, kind="ExternalInput")
    NQK = nc.dram_tensor("NQK", [N_POOL, D], BF16, kind="ExternalInput")
    NV = nc.dram_tensor("NV", [N_POOL, D], F32R, kind="ExternalInput")
    GQT = nc.dram_tensor("GQT", [N_POOL, TOK], BF16, kind="ExternalInput")
    GKT = nc.dram_tensor("GKT", [N_POOL, TOK], BF16, kind="ExternalInput")
    GVT = nc.dram_tensor("GVT", [N_POOL, TOK], F32, kind="ExternalInput")
    MASKS = nc.dram_tensor("MASKS", [ST, 128, TOK], F32R, kind="ExternalInput")
    ONES = nc.dram_tensor("ONES", [128, 128], F32R, kind="ExternalInput")
    IDN = nc.dram_tensor("IDN", [128, 128], F32R, kind="ExternalInput")
    WO = nc.dram_tensor("WO", [D, D], F32R, kind="ExternalInput")
    OT = nc.dram_tensor("OT", [D, TOK], F32, kind="ExternalOutput")

    # ---- collective staging -------------------------------------------
    kt_stage = nc.dram_tensor("kt_stage", [D, TOK], BF16)
    kt_gath = nc.dram_tensor("kt_gath", [2 * D, TOK], BF16)
    v_stage = nc.dram_tensor("v_stage", [TOK, D], F32R)
    v_gath = nc.dram_tensor("v_gath", [S, D], F32R)

    with tile.TileContext(nc) as tc:
        with tc.tile_pool(name="qkt", bufs=1) as p_qkt:
            with tc.tile_pool(name="base", bufs=1) as p_base, \
                 tc.tile_pool(name="strm", bufs=1) as p_st:
                # x^T chunks (resident through routing)
                xt_sb = []
                for kc in range(DC):
                    t = p_base.tile([128, TOK], BF16, name=f"xt{kc}",
                                    tag=f"xt{kc}")
                    nc.sync.dma_start(out=t[:],
                                      in_=XT[kc * 128:(kc + 1) * 128, :])
                    xt_sb.append(t)

                # ---- QK pool: route + recon + K^T exchange -------------
                with tc.tile_pool(name="rqk", bufs=1) as p_rqk:
                    with tc.tile_pool(name="ps_rt_qk", bufs=1,
                                      space="PSUM") as ps_rt:
                        rq_sb, rk_sb = _route(
                            nc, p_st, p_rqk, ps_rt, xt_sb, NTQKB,
                            [(GQT, BF16, BF16, "q"), (GKT, BF16, BF16, "k")])
                    with tc.tile_pool(name="ps_acc_qk", bufs=1,
                                      space="PSUM") as ps_acc:
                        qt_sb = _recon_T(nc, p_st, p_qkt, ps_acc,
                                         NQK, rq_sb, BF16, BF16, "qt")
                        kt_sb = _recon_T(nc, p_st, p_rqk, ps_acc,
                                         NQK, rk_sb, BF16, BF16, "kt")
                    for dt in range(DC):
                        nc.sync.dma_start(
                            out=kt_stage[dt * 128:(dt + 1) * 128, :],
                            in_=kt_sb[dt][:])
                    nc.gpsimd.collective_compute(
                        "AllGather", mybir.AluOpType.bypass,
                        replica_groups=REPLICA_GROUPS,
                        ins=[kt_stage[:]], outs=[kt_gath[:]],
                    )

                # ---- V pool: route + recon + V exchange ----------------
                with tc.tile_pool(name="rv", bufs=1) as p_rv:
                    with tc.tile_pool(name="ps_rt_v", bufs=1,
                                      space="PSUM") as ps_rt_v:
                        xtf_sb = []
                        for kc in range(DC):
                            t = p_rv.tile([128, TOK], F32R,
                                          name=f"xtf{kc}", tag=f"xtf{kc}")
                            nc.sync.dma_start(
                                out=t[:],
                                in_=XTF[kc * 128:(kc + 1) * 128, :])
                            xtf_sb.append(t)
                        (rv_sb,) = _route(
                            nc, p_st, p_rv, ps_rt_v, xtf_sb, NTVB,
                            [(GVT, F32, F32R, "v")], ntb_dt=F32R)
                    with tc.tile_pool(name="ps_acc_v", bufs=1,
                                      space="PSUM") as ps_acc_v:
                        v_acc = [ps_acc_v.tile([128, 512], F32,
                                               name=f"vacc{i}", tag=f"vacc{i}")
                                 for i in range(2 * TT)]
                        for pc in range(PC):
                            nvch = p_st.tile([128, D], F32R, name=f"nvch_{pc}",
                                             tag="nvchunk", bufs=4)
                            nc.sync.dma_start(
                                out=nvch[:],
                                in_=NV[pc * 128:(pc + 1) * 128, :])
                            for t in range(TT):
                                for dh in range(2):
                                    nc.tensor.matmul(
                                        v_acc[t * 2 + dh][:],
                                        rv_sb[pc][:, t * 128:(t + 1) * 128],
                                        nvch[:, dh * 512:(dh + 1) * 512],
                                        start=(pc == 0),
                                        stop=(pc == PC - 1),
                                    )
                        for t in range(TT):
                            for dh in range(2):
                                o = p_rv.tile([128, 512], F32R,
                                              name=f"vsb{t}_{dh}",
                                              tag=f"vsb{t}_{dh}")
                                nc.scalar.copy(o[:], v_acc[t * 2 + dh][:])
                                nc.sync.dma_start(
                                    out=v_stage[t * 128:(t + 1) * 128,
                                                dh * 512:(dh + 1) * 512],
                                    in_=o[:])
                    nc.gpsimd.collective_compute(
                        "AllGather", mybir.AluOpType.bypass,
                        replica_groups=REPLICA_GROUPS,
                        ins=[v_stage[:]], outs=[v_gath[:]],
                    )

            # ================= attention ================================
            with tc.tile_pool(name="att", bufs=1) as p_att, \
                 tc.tile_pool(name="attw", bufs=1) as p_attw:
                kt_att = []
                for u in range(DC):
                    t = p_att.tile([128, S], BF16, name=f"ktatt{u}",
                                   tag=f"ktatt{u}")
                    for g in range(2):
                        nc.sync.dma_start(
                            out=t[:, g * TOK:(g + 1) * TOK],
                            in_=kt_gath[g * D + u * 128:
                                        g * D + (u + 1) * 128, :])
                    kt_att.append(t)
                v_att = []
                for i in range(ST):
                    t = p_att.tile([128, D], F32R, name=f"vatt{i}",
                                   tag=f"vatt{i}")
                    nc.sync.dma_start(out=t[:],
                                      in_=v_gath[i * 128:(i + 1) * 128, :])
                    v_att.append(t)
                mask_sb = []
                for i in range(ST):
                    t = p_att.tile([128, TOK], F32R, name=f"mask{i}",
                                   tag=f"mask{i}")
                    nc.sync.dma_start(out=t[:], in_=MASKS[i])
                    mask_sb.append(t)
                ones_sb = p_att.tile([128, 128], F32R, name="ones", tag="ones")
                nc.sync.dma_start(out=ones_sb[:], in_=ONES[:])
                idn_sb = p_att.tile([128, 128], F32R, name="idn", tag="idn")
                nc.sync.dma_start(out=idn_sb[:], in_=IDN[:])
                attn_sb = [p_att.tile([128, TOK], F32R, name=f"attn{u}",
                                      tag=f"attn{u}") for u in range(DC)]

                with tc.tile_pool(name="ps_att", bufs=1,
                                  space="PSUM") as ps_att:
                    for u in range(DC):
                        # ---- head pair hg = 2u (par=0), 2u+1 (par=1) ----
                        # prefetch [V_h | ones] stationary tiles
                        vo_t = {}
                        for par in range(2):
                            hg = 2 * u + par
                            for i in range(ST):
                                vo = p_attw.tile([128, 65], F32R,
                                                 name=f"vo_{hg}_{i}",
                                                 tag="vo", bufs=20)
                                nc.vector.tensor_copy(
                                    vo[:, 0:64],
                                    v_att[i][:, hg * 64:(hg + 1) * 64])
                                nc.vector.tensor_copy(vo[:, 64:65],
                                                      ones_sb[:, 0:1])
                                vo_t[(par, i)] = vo
                        # scores^T blocks with additive causal mask, exp'd
                        a_t = {}
                        for i in range(ST):
                            for par in range(2):
                                hg = 2 * u + par
                                p0 = 64 * par
                                ps_s = ps_att.tile([128, TOK], F32,
                                                   name=f"pss_{hg}_{i}",
                                                   tag="ps_s", bufs=4)
                                nc.tensor.matmul(
                                    ps_s[:], idn_sb[:], mask_sb[i][:],
                                    start=True, stop=False,
                                    skip_group_check=True)
                                nc.tensor.matmul(
                                    ps_s[:],
                                    kt_att[u][p0:p0 + 64,
                                              i * 128:(i + 1) * 128],
                                    qt_sb[u][p0:p0 + 64, :],
                                    start=False, stop=True,
                                    skip_group_check=True)
                                a = p_attw.tile([128, TOK], F32R,
                                                name=f"a_{hg}_{i}",
                                                tag="asb", bufs=18)
                                nc.scalar.activation(
                                    a[:], ps_s[:],
                                    mybir.ActivationFunctionType.Exp,
                                    scale=float(1.0 / np.sqrt(DH)))
                                a_t[(par, i)] = a
                        # attn-out + denominator; normalize
                        for par in range(2):
                            hg = 2 * u + par
                            p0 = 64 * par
                            ps_o = ps_att.tile([65, TOK], F32,
                                               name=f"pso_{hg}",
                                               tag="ps_o", bufs=2)
                            for i in range(ST):
                                nc.tensor.matmul(
                                    ps_o[:], vo_t[(par, i)][:],
                                    a_t[(par, i)][:],
                                    start=(i == 0), stop=(i == ST - 1),
                                )
                            linv = p_attw.tile([128, TOK], F32R,
                                               name=f"linv{hg}",
                                               tag="linv", bufs=2)
                            with nc.allow_low_precision(
                                    reason="f32r is bit-identical to f32"):
                                nc.vector.reciprocal(linv[64:65, :],
                                                     ps_o[64:65, :])
                            ps_b = ps_att.tile([128, TOK], F32,
                                               name=f"psb_{hg}",
                                               tag="ps_b", bufs=2)
                            nc.tensor.matmul(
                                ps_b[:], ones_sb[64:65, :], linv[64:65, :],
                                start=True, stop=True)
                            binv = p_attw.tile([128, TOK], F32R,
                                               name=f"binv{hg}",
                                               tag="binv", bufs=2)
                            nc.scalar.copy(binv[:], ps_b[:])
                            if p0 == 0:
                                nc.vector.tensor_mul(
                                    attn_sb[u][0:64, :], ps_o[0:64, :],
                                    binv[0:64, :])
                            else:
                                tmp = p_attw.tile([64, TOK], F32R,
                                                  name=f"atmp{hg}",
                                                  tag="atmp", bufs=2)
                                nc.vector.tensor_mul(tmp[:], ps_o[0:64, :],
                                                     binv[0:64, :])
                                nc.sync.dma_start(
                                    out=attn_sb[u][64:128, :], in_=tmp[:])

                # ---- W_O (weights streamed) ----------------------------
                with tc.tile_pool(name="ps_wo", bufs=1,
                                  space="PSUM") as ps_wo:
                    for dt in range(DC):
                        ps = ps_wo.tile([128, TOK], F32, name=f"psot{dt}",
                                        tag="ps_ot", bufs=2)
                        for dc in range(DC):
                            w = p_attw.tile([128, 128], F32R,
                                            name=f"wo_{dt}_{dc}",
                                            tag="wo", bufs=4)
                            nc.sync.dma_start(
                                out=w[:],
                                in_=WO[dc * 128:(dc + 1) * 128,
                                       dt * 128:(dt + 1) * 128])
                            nc.tensor.matmul(
                                ps[:], w[:], attn_sb[dc][:],
                                start=(dc == 0), stop=(dc == DC - 1),
                            )
                        o = p_attw.tile([128, TOK], F32, name=f"ot{dt}",
                                        tag="otsb", bufs=3)
                        nc.scalar.copy(o[:], ps[:])
                        nc.sync.dma_start(
                            out=OT[dt * 128:(dt + 1) * 128, :], in_=o[:])

    nc.compile()
    return nc


def _build_inputs(inputs):
    x = np.asarray(inputs["x"], np.float32)
    g_Q = np.asarray(inputs["g_Q"], np.float32)
    g_K = np.asarray(inputs["g_K"], np.float32)
    g_V = np.asarray(inputs["g_V"], np.float32)
    ci_qk = np.asarray(inputs["ci_qk"])
    ci_v = np.asarray(inputs["ci_v"])
    nqk = np.asarray(inputs["qk_neurons"], np.float32)
    nv = np.asarray(inputs["v_neurons"], np.float32)
    wo = np.asarray(inputs["W_O"], np.float32)
    bf = ml_dtypes.bfloat16

    # Pool blocks for P^T: NTB[m][p, kc*128 + j] = N[m*128 + j, kc*128 + p]
    def pool_blocks(n):
        v = n.reshape(PC, 128, DC, 128)                     # [m, j, kc, p]
        return np.ascontiguousarray(
            v.transpose(0, 3, 2, 1).reshape(PC, 128, D))    # [m, p, (kc j)]

    ntqkb = pool_blocks(nqk).astype(bf)
    ntvb = pool_blocks(nv)
    nqk_bf = nqk.astype(bf)

    def gate_T(g_b, ci_b):
        # [N_POOL, TOK]: G^T[p, t] = sum_{n: ci[t,n]=p} g[t,n]
        out = np.zeros((N_POOL, TOK), np.float32)
        t_idx = np.repeat(np.arange(TOK), K_SEL)
        np.add.at(out, (ci_b.ravel(), t_idx), g_b.ravel())
        return out

    in_maps = []
    for c in range(N_CORES):
        b, h = c // 2, c % 2
        sl = slice(h * TOK, (h + 1) * TOK)
        masks = np.zeros((ST, 128, TOK), np.float32)
        s_glob = h * TOK + np.arange(TOK)[None, :]
        for i in range(ST):
            t_glob = i * 128 + np.arange(128)[:, None]
            masks[i] = np.where(t_glob <= s_glob, 0.0, -30.0 * np.sqrt(DH)
                                ).astype(np.float32)
        in_maps.append({
            "XT": np.ascontiguousarray(x[b, sl, :].T).astype(bf),
            "XTF": np.ascontiguousarray(x[b, sl, :].T),
            "NTQKB": ntqkb,
            "NTVB": ntvb,
            "NQK": nqk_bf,
            "NV": nv,
            "GQT": gate_T(g_Q[b, sl], ci_qk[b, sl]).astype(bf),
            "GKT": gate_T(g_K[b, sl], ci_qk[b, sl]).astype(bf),
            "GVT": gate_T(g_V[b, sl], ci_v[b, sl]),
            "MASKS": masks,
            "ONES": np.ones((128, 128), np.float32),
            "IDN": np.eye(128, dtype=np.float32),
            "WO": wo,
        })
    return in_maps


def kernel(**inputs) -> np.ndarray:
    if "nc" not in _CACHE:
        _CACHE["nc"] = _build_nc()
    nc = _CACHE["nc"]
    in_maps = _build_inputs(inputs)

    trace = bool(int(os.environ.get("BASS_KERNEL_TRACE", "0")))
    res = run_bass_kernel_spmd(nc, in_maps, list(range(N_CORES)), trace=trace)
    if trace and res.exec_time_ns is not None:
        print(f"HW exec time: {res.exec_time_ns} ns")

    out = np.zeros((B, S, D), np.float32)
    for c in range(N_CORES):
        b, h = c // 2, c % 2
        ot = res.results[c]["OT"]  # [D, TOK]
        out[b, h * TOK:(h + 1) * TOK, :] = np.asarray(ot, np.float32).T
    return out


# revision 14
# speedup vs baseline: 1.1721x; 1.1721x over previous
"""Trainium2 Bass kernel for nn_AttentionCircuit (moe_routing).

Math (per batch b):
  P_qk = x_b @ qk_neurons.T            [S, NPOOL]   (dense "router" matmul)
  act[s,n] = P_qk[s, ci_qk[s,n]]
  Q = sum_n (act*gQ)[s,n] * qk_neurons[ci_qk[s,n]]  (ditto K with gK, V w/ v pool)
  causal MHA (H=16, dh=64) + W_O

Key identity: with G[s,p] = sum_{n: ci[s,n]=p} g[s,n] (host-built scatter of
the gates) the gathered reconstruction collapses to dense algebra:
  Q = (P ⊙ G_Q) @ N        =>   Q^T = N^T @ (P^T ⊙ G_Q^T)
so the MoE routing becomes two dense matmuls + one elementwise gate, with
P^T = N @ x^T computed directly in pool-major layout (no device transposes,
no gather/scatter instructions; duplicate indices handled by the host sum).

Sharding: 8 cores = (batch b = c//2) x (sequence half h = c%2). Each core:
  - routes its own 512 tokens (P^T, R^T = P^T ⊙ G^T, Q^T/K^T/V recon)
  - AllGathers K^T and V across its pair (same batch)
  - computes causal attention for its 512 queries over all 16 heads
    (causality via per-core host-built additive -inf masks pre-loaded into
     the scores PSUM by an identity matmul; the softmax denominator comes
     free from a [V_h | ones] stationary operand)
  - applies W_O and writes out^T [D, 512] for its tokens.

dtypes: QK routing in bf16 (errors only perturb tiny attention scores);
V path and W_O entirely in fp32r (full matmul rate, ~1.6e-4 error on HW).
"""

import os
import numpy as np
import ml_dtypes

import concourse.mybir as mybir
import concourse.tile as tile
from concourse import bacc
from concourse.bass_utils import run_bass_kernel_spmd

B, S, D = 4, 1024, 1024
H = 16
K_SEL = 32
N_POOL = 4096
N_CORES = 8
TOK = 512           # tokens per core
DH = D // H         # 64
PC = N_POOL // 128  # 32 pool chunks
DC = D // 128       # 8 feature chunks
TT = TOK // 128     # 4 token tiles
ST = S // 128       # 8 key tiles

BF16 = mybir.dt.bfloat16
F32 = mybir.dt.float32
F32R = mybir.dt.float32r

REPLICA_GROUPS = [[0, 1], [2, 3], [4, 5], [6, 7]]

_CACHE = {}


def _build_nc():
    nc = bacc.Bacc("TRN2", target_bir_lowering=False, debug=False,
                   num_devices=N_CORES)

    # ---- per-core external inputs -------------------------------------
    XT = nc.dram_tensor("XT", [D, TOK], BF16, kind="ExternalInput")
    XTF = nc.dram_tensor("XTF", [D, TOK], F32R, kind="ExternalInput")
    NTQKB = nc.dram_tensor("NTQKB", [PC, 128, D], BF16, kind="ExternalInput")
    NTVB = nc.dram_tensor("NTVB", [PC, 128, D], F32R, kind="ExternalInput")
    NQK = nc.dram_tensor("NQK", [N_POOL, D], BF16, kind="ExternalInput")
    NV = nc.dram_tensor("NV", [N_POOL, D], F32R, kind="ExternalInput")
    GQKT = nc.dram_tensor("GQKT", [N_POOL, 2 * TOK], BF16, kind="ExternalInput")
    GVT = nc.dram_tensor("GVT", [N_POOL, TOK], F32, kind="ExternalInput")
    MASKS = nc.dram_tensor("MASKS", [128, ST * TOK], F32R, kind="ExternalInput")
    ONES = nc.dram_tensor("ONES", [128, 128], F32R, kind="ExternalInput")
    IDN = nc.dram_tensor("IDN", [128, 128], F32R, kind="ExternalInput")
    WO = nc.dram_tensor("WO", [D, D], F32R, kind="ExternalInput")
    OT = nc.dram_tensor("OT", [D, TOK], F32, kind="ExternalOutput")

    # ---- collective staging -------------------------------------------
    kt_stage = nc.dram_tensor("kt_stage", [D, TOK], BF16)
    kt_gath = nc.dram_tensor("kt_gath", [2 * D, TOK], BF16)
    v_stage = nc.dram_tensor("v_stage", [TOK, D], F32R)
    v_gath = nc.dram_tensor("v_gath", [S, D], F32R)

    with tile.TileContext(nc) as tc:
        with (
            tc.tile_pool(name="qkt", bufs=1) as p_qkt,      # Q^T, resident
            tc.tile_pool(name="atte", bufs=1) as p_ae,      # early attn loads
        ):
            # masks / ones / identity: no deps, load first
            mask_all = p_ae.tile([128, ST * TOK], F32R, name="mask_all",
                                 tag="mask_all")
            nc.sync.dma_start(out=mask_all[:], in_=MASKS[:])
            ones_sb = p_ae.tile([128, 128], F32R, name="ones", tag="ones")
            nc.sync.dma_start(out=ones_sb[:], in_=ONES[:])
            idn_sb = p_ae.tile([128, 128], F32R, name="idn", tag="idn")
            nc.sync.dma_start(out=idn_sb[:], in_=IDN[:])
            kt_att = [p_ae.tile([128, S], BF16, name=f"ktatt{u}",
                                tag=f"ktatt{u}") for u in range(DC)]

            # =========== QK pool: route + joint recon ===================
            with tc.tile_pool(name="rqk", bufs=1) as p_rqk, \
                 tc.tile_pool(name="strmqk", bufs=1) as p_sq:
                xt_sb = []
                for kc in range(DC):
                    t = p_rqk.tile([128, TOK], BF16, name=f"xt{kc}",
                                   tag=f"xt{kc}")
                    nc.sync.dma_start(out=t[:],
                                      in_=XT[kc * 128:(kc + 1) * 128, :])
                    xt_sb.append(t)

                rq_sb, rk_sb = [], []
                with tc.tile_pool(name="ps_rt_qk", bufs=1,
                                  space="PSUM") as ps_rt:
                    for m in range(PC):
                        ntb = p_sq.tile([128, D], BF16, name=f"ntbq{m}",
                                        tag="ntbq", bufs=4)
                        nc.sync.dma_start(out=ntb[:], in_=NTQKB[m])
                        pt = ps_rt.tile([128, TOK], F32, name=f"ptq{m}",
                                        tag="pt", bufs=3)
                        for kc in range(DC):
                            nc.tensor.matmul(
                                pt[:], ntb[:, kc * 128:(kc + 1) * 128],
                                xt_sb[kc][:],
                                start=(kc == 0), stop=(kc == DC - 1))
                        gqk = p_sq.tile([128, 2 * TOK], BF16, name=f"gqk{m}",
                                        tag="gqk", bufs=4)
                        nc.sync.dma_start(
                            out=gqk[:], in_=GQKT[m * 128:(m + 1) * 128, :])
                        rq = p_rqk.tile([128, TOK], BF16, name=f"rq{m}",
                                        tag=f"rq{m}")
                        nc.vector.tensor_mul(rq[:], pt[:], gqk[:, 0:TOK])
                        rk = p_rqk.tile([128, TOK], BF16, name=f"rk{m}",
                                        tag=f"rk{m}")
                        nc.vector.tensor_mul(rk[:], pt[:], gqk[:, TOK:2 * TOK])
                        rq_sb.append(rq)
                        rk_sb.append(rk)

                # joint Q^T/K^T recon: two half-D passes over NQK
                qt_sb = [p_qkt.tile([128, TOK], BF16, name=f"qt{dt}",
                                    tag=f"qt{dt}") for dt in range(DC)]
                kt_sb = [p_rqk.tile([128, TOK], BF16, name=f"kt{dt}",
                                    tag=f"kt{dt}") for dt in range(DC)]
                with tc.tile_pool(name="ps_acc_qk", bufs=1,
                                  space="PSUM") as ps_acc:
                    for half in range(2):
                        acc_q = [ps_acc.tile([128, TOK], F32,
                                             name=f"aq{half}_{j}",
                                             tag=f"aq{j}") for j in range(4)]
                        acc_k = [ps_acc.tile([128, TOK], F32,
                                             name=f"ak{half}_{j}",
                                             tag=f"ak{j}") for j in range(4)]
                        for pc in range(PC):
                            nq = p_sq.tile([128, 512], BF16,
                                           name=f"nq{half}_{pc}",
                                           tag="nqh", bufs=4)
                            nc.sync.dma_start(
                                out=nq[:],
                                in_=NQK[pc * 128:(pc + 1) * 128,
                                        half * 512:(half + 1) * 512])
                            for j in range(4):
                                nc.tensor.matmul(
                                    acc_q[j][:], nq[:, j * 128:(j + 1) * 128],
                                    rq_sb[pc][:],
                                    start=(pc == 0), stop=(pc == PC - 1))
                            for j in range(4):
                                nc.tensor.matmul(
                                    acc_k[j][:], nq[:, j * 128:(j + 1) * 128],
                                    rk_sb[pc][:],
                                    start=(pc == 0), stop=(pc == PC - 1))
                        for j in range(4):
                            dt = half * 4 + j
                            nc.scalar.copy(qt_sb[dt][:], acc_q[j][:])
                            nc.scalar.copy(kt_sb[dt][:], acc_k[j][:])
                for dt in range(DC):
                    nc.sync.dma_start(
                        out=kt_stage[dt * 128:(dt + 1) * 128, :],
                        in_=kt_sb[dt][:])
                nc.gpsimd.collective_compute(
                    "AllGather", mybir.AluOpType.bypass,
                    replica_groups=REPLICA_GROUPS,
                    ins=[kt_stage[:]], outs=[kt_gath[:]],
                )
            # K^T for all S keys (early, before the V phase queues DMAs)
            for u in range(DC):
                for g in range(2):
                    nc.sync.dma_start(
                        out=kt_att[u][:, g * TOK:(g + 1) * TOK],
                        in_=kt_gath[g * D + u * 128: g * D + (u + 1) * 128, :])

            # =========== V pool: route + recon + exchange ===============
            with tc.tile_pool(name="rv", bufs=1) as p_rv, \
                 tc.tile_pool(name="strmv", bufs=1) as p_sv:
                xtf_sb = []
                for kc in range(DC):
                    t = p_rv.tile([128, TOK], F32R, name=f"xtf{kc}",
                                  tag=f"xtf{kc}")
                    nc.sync.dma_start(out=t[:],
                                      in_=XTF[kc * 128:(kc + 1) * 128, :])
                    xtf_sb.append(t)
                rv_sb = []
                with tc.tile_pool(name="ps_rt_v", bufs=1,
                                  space="PSUM") as ps_rt_v:
                    for m in range(PC):
                        ntb = p_sv.tile([128, D], F32R, name=f"ntbv{m}",
                                        tag="ntbv", bufs=4)
                        nc.sync.dma_start(out=ntb[:], in_=NTVB[m])
                        pt = ps_rt_v.tile([128, TOK], F32, name=f"ptv{m}",
                                          tag="pt", bufs=3)
                        for kc in range(DC):
                            nc.tensor.matmul(
                                pt[:], ntb[:, kc * 128:(kc + 1) * 128],
                                xtf_sb[kc][:],
                                start=(kc == 0), stop=(kc == DC - 1))
                        gv = p_sv.tile([128, TOK], F32, name=f"gv{m}",
                                       tag="gv", bufs=4)
                        nc.sync.dma_start(
                            out=gv[:], in_=GVT[m * 128:(m + 1) * 128, :])
                        rv = p_rv.tile([128, TOK], F32R, name=f"rv{m}",
                                       tag=f"rv{m}")
                        nc.vector.tensor_mul(rv[:], pt[:], gv[:])
                        rv_sb.append(rv)

                with tc.tile_pool(name="ps_acc_v", bufs=1,
                                  space="PSUM") as ps_acc_v:
                    v_acc = [ps_acc_v.tile([128, 512], F32, name=f"vacc{i}",
                                           tag=f"vacc{i}")
                             for i in range(2 * TT)]
                    for pc in range(PC):
                        nvch = p_sv.tile([128, D], F32R, name=f"nvch{pc}",
                                         tag="nvchunk", bufs=4)
                        nc.sync.dma_start(
                            out=nvch[:], in_=NV[pc * 128:(pc + 1) * 128, :])
                        for t in range(TT):
                            for dh in range(2):
                                nc.tensor.matmul(
                                    v_acc[t * 2 + dh][:],
                                    rv_sb[pc][:, t * 128:(t + 1) * 128],
                                    nvch[:, dh * 512:(dh + 1) * 512],
                                    start=(pc == 0), stop=(pc == PC - 1))
                    for t in range(TT):
                        for dh in range(2):
                            o = p_rv.tile([128, 512], F32R,
                                          name=f"vsb{t}_{dh}",
                                          tag=f"vsb{t}_{dh}")
                            nc.scalar.copy(o[:], v_acc[t * 2 + dh][:])
                            nc.sync.dma_start(
                                out=v_stage[t * 128:(t + 1) * 128,
                                            dh * 512:(dh + 1) * 512],
                                in_=o[:])
                nc.gpsimd.collective_compute(
                    "AllGather", mybir.AluOpType.bypass,
                    replica_groups=REPLICA_GROUPS,
                    ins=[v_stage[:]], outs=[v_gath[:]],
                )

            # ================= attention + W_O ==========================
            with tc.tile_pool(name="att", bufs=1) as p_att, \
                 tc.tile_pool(name="attw", bufs=1) as p_attw:
                # W_O resident (loads overlap V recon tail)
                wo_sb = []
                for dc in range(DC):
                    t = p_att.tile([128, D], F32R, name=f"wo{dc}",
                                   tag=f"wo{dc}")
                    nc.sync.dma_start(out=t[:],
                                      in_=WO[dc * 128:(dc + 1) * 128, :])
                    wo_sb.append(t)
                # V with interleaved [V_h | 1] layout: voall[i][:, hg*65:+65]
                v_att = []
                vo_all = []
                for i in range(ST):
                    t = p_att.tile([128, D], F32R, name=f"vatt{i}",
                                   tag=f"vatt{i}")
                    nc.sync.dma_start(out=t[:],
                                      in_=v_gath[i * 128:(i + 1) * 128, :])
                    v_att.append(t)
                    va = p_att.tile([128, H * 65], F32R, name=f"voall{i}",
                                    tag=f"voall{i}")
                    dst = va[:].rearrange("p (h c) -> p h c", c=65)
                    src = t[:].rearrange("p (h c) -> p h c", c=64)
                    nc.vector.tensor_copy(dst[:, :, 0:64], src[:])
                    nc.vector.tensor_copy(
                        dst[:, :, 64:65],
                        ones_sb[:, 0:H].rearrange("p (h c) -> p h c", c=1))
                    vo_all.append(va)

                attn_sb = [p_att.tile([128, TOK], F32R, name=f"attn{u}",
                                      tag=f"attn{u}") for u in range(DC)]

                with tc.tile_pool(name="ps_att", bufs=1,
                                  space="PSUM") as ps_att:
                    for u in range(DC):
                        a_t = {}
                        for i in range(ST):
                            for par in range(2):
                                hg = 2 * u + par
                                p0 = 64 * par
                                ps_s = ps_att.tile([128, TOK], F32,
                                                   name=f"pss_{hg}_{i}",
                                                   tag="ps_s", bufs=4)
                                nc.tensor.matmul(
                                    ps_s[:], idn_sb[:],
                                    mask_all[:, i * TOK:(i + 1) * TOK],
                                    start=True, stop=False,
                                    skip_group_check=True)
                                nc.tensor.matmul(
                                    ps_s[:],
                                    kt_att[u][p0:p0 + 64,
                                              i * 128:(i + 1) * 128],
                                    qt_sb[u][p0:p0 + 64, :],
                                    start=False, stop=True,
                                    skip_group_check=True)
                                a = p_attw.tile([128, TOK], F32R,
                                                name=f"a_{hg}_{i}",
                                                tag="asb", bufs=18)
                                nc.scalar.activation(
                                    a[:], ps_s[:],
                                    mybir.ActivationFunctionType.Exp,
                                    scale=float(1.0 / np.sqrt(DH)))
                                a_t[(par, i)] = a
                        for par in range(2):
                            hg = 2 * u + par
                            p0 = 64 * par
                            ps_o = ps_att.tile([65, TOK], F32,
                                               name=f"pso_{hg}",
                                               tag="ps_o", bufs=2)
                            for i in range(ST):
                                nc.tensor.matmul(
                                    ps_o[:],
                                    vo_all[i][:, hg * 65:(hg + 1) * 65],
                                    a_t[(par, i)][:],
                                    start=(i == 0), stop=(i == ST - 1))
                            # denominator broadcast + normalize
                            lsb = p_attw.tile([128, TOK], F32R,
                                              name=f"lsb{hg}",
                                              tag="lsb", bufs=2)
                            nc.scalar.copy(lsb[64:65, :], ps_o[64:65, :])
                            ps_b = ps_att.tile([128, TOK], F32,
                                               name=f"psb_{hg}",
                                               tag="ps_b", bufs=2)
                            nc.tensor.matmul(
                                ps_b[:], ones_sb[64:65, :], lsb[64:65, :],
                                start=True, stop=True)
                            binv = p_attw.tile([128, TOK], F32R,
                                               name=f"binv{hg}",
                                               tag="binv", bufs=2)
                            with nc.allow_low_precision(
                                    reason="f32r is bit-identical to f32"):
                                nc.vector.reciprocal(binv[:], ps_b[:])
                            if p0 == 0:
                                nc.vector.tensor_mul(
                                    attn_sb[u][0:64, :], ps_o[0:64, :],
                                    binv[0:64, :])
                            else:
                                tmp = p_attw.tile([64, TOK], F32R,
                                                  name=f"atmp{hg}",
                                                  tag="atmp", bufs=2)
                                nc.vector.tensor_mul(tmp[:], ps_o[0:64, :],
                                                     binv[0:64, :])
                                nc.sync.dma_start(
                                    out=attn_sb[u][64:128, :], in_=tmp[:])

                # ---- W_O ----------------------------------------------
                with tc.tile_pool(name="ps_wo", bufs=1,
                                  space="PSUM") as ps_wo:
                    for dt in range(DC):
                        ps = ps_wo.tile([128, TOK], F32, name=f"psot{dt}",
                                        tag="ps_ot", bufs=2)
                        for dc in range(DC):
                            nc.tensor.matmul(
                                ps[:],
                                wo_sb[dc][:, dt * 128:(dt + 1) * 128],
                                attn_sb[dc][:],
                                start=(dc == 0), stop=(dc == DC - 1))
                        o = p_attw.tile([128, TOK], F32, name=f"ot{dt}",
                                        tag="otsb", bufs=3)
                        nc.scalar.copy(o[:], ps[:])
                        nc.sync.dma_start(
                            out=OT[dt * 128:(dt + 1) * 128, :], in_=o[:])

    nc.compile()
    return nc


def _build_inputs(inputs):
    x = np.asarray(inputs["x"], np.float32)
    g_Q = np.asarray(inputs["g_Q"], np.float32)
    g_K = np.asarray(inputs["g_K"], np.float32)
    g_V = np.asarray(inputs["g_V"], np.float32)
    ci_qk = np.asarray(inputs["ci_qk"])
    ci_v = np.asarray(inputs["ci_v"])
    nqk = np.asarray(inputs["qk_neurons"], np.float32)
    nv = np.asarray(inputs["v_neurons"], np.float32)
    wo = np.asarray(inputs["W_O"], np.float32)
    bf = ml_dtypes.bfloat16

    # Pool blocks for P^T: NTB[m][p, kc*128 + j] = N[m*128 + j, kc*128 + p]
    def pool_blocks(n):
        v = n.reshape(PC, 128, DC, 128)                     # [m, j, kc, p]
        return np.ascontiguousarray(
            v.transpose(0, 3, 2, 1).reshape(PC, 128, D))    # [m, p, (kc j)]

    ntqkb = pool_blocks(nqk).astype(bf)
    ntvb = pool_blocks(nv)
    nqk_bf = nqk.astype(bf)

    def gate_T(g_b, ci_b):
        # [N_POOL, TOK]: G^T[p, t] = sum_{n: ci[t,n]=p} g[t,n]
        out = np.zeros((N_POOL, TOK), np.float32)
        t_idx = np.repeat(np.arange(TOK), K_SEL)
        np.add.at(out, (ci_b.ravel(), t_idx), g_b.ravel())
        return out

    in_maps = []
    for c in range(N_CORES):
        b, h = c // 2, c % 2
        sl = slice(h * TOK, (h + 1) * TOK)
        masks = np.zeros((128, ST * TOK), np.float32)
        s_glob = h * TOK + np.arange(TOK)[None, :]
        for i in range(ST):
            t_glob = i * 128 + np.arange(128)[:, None]
            masks[:, i * TOK:(i + 1) * TOK] = np.where(
                t_glob <= s_glob, 0.0, -30.0 * np.sqrt(DH))
        gq = gate_T(g_Q[b, sl], ci_qk[b, sl]).astype(bf)
        gk = gate_T(g_K[b, sl], ci_qk[b, sl]).astype(bf)
        in_maps.append({
            "XT": np.ascontiguousarray(x[b, sl, :].T).astype(bf),
            "XTF": np.ascontiguousarray(x[b, sl, :].T),
            "NTQKB": ntqkb,
            "NTVB": ntvb,
            "NQK": nqk_bf,
            "NV": nv,
            "GQKT": np.concatenate([gq, gk], axis=1),
            "GVT": gate_T(g_V[b, sl], ci_v[b, sl]),
            "MASKS": masks,
            "ONES": np.ones((128, 128), np.float32),
            "IDN": np.eye(128, dtype=np.float32),
            "WO": wo,
        })
    return in_maps


def kernel(**inputs) -> np.ndarray:
    if "nc" not in _CACHE:
        _CACHE["nc"] = _build_nc()
    nc = _CACHE["nc"]
    in_maps = _build_inputs(inputs)

    trace = bool(int(os.environ.get("BASS_KERNEL_TRACE", "0")))
    res = run_bass_kernel_spmd(nc, in_maps, list(range(N_CORES)), trace=trace)
    if trace and res.exec_time_ns is not None:
        print(f"HW exec time: {res.exec_time_ns} ns")

    out = np.zeros((B, S, D), np.float32)
    for c in range(N_CORES):
        b, h = c // 2, c % 2
        ot = res.results[c]["OT"]  # [D, TOK]
        out[b, h * TOK:(h + 1) * TOK, :] = np.asarray(ot, np.float32).T
    return out


# revision 15
# speedup vs baseline: 1.1966x; 1.0209x over previous
"""Trainium2 Bass kernel for nn_AttentionCircuit (moe_routing).

Math (per batch b):
  P_qk = x_b @ qk_neurons.T            [S, NPOOL]   (dense "router" matmul)
  act[s,n] = P_qk[s, ci_qk[s,n]]
  Q = sum_n (act*gQ)[s,n] * qk_neurons[ci_qk[s,n]]  (ditto K with gK, V w/ v pool)
  causal MHA (H=16, dh=64) + W_O

Key identity: with G[s,p] = sum_{n: ci[s,n]=p} g[s,n] (host-built scatter of
the gates) the gathered reconstruction collapses to dense algebra:
  Q = (P ⊙ G_Q) @ N        =>   Q^T = N^T @ (P^T ⊙ G_Q^T)
so the MoE routing becomes two dense matmuls + one elementwise gate, with
P^T = N @ x^T computed directly in pool-major layout (no device transposes,
no gather/scatter instructions; duplicate indices handled by the host sum).

Sharding: 8 cores = (batch b = c//2) x (sequence half h = c%2). Each core:
  - routes its own 512 tokens (P^T, R^T = P^T ⊙ G^T, Q^T/K^T/V recon)
  - AllGathers K^T and V across its pair (same batch)
  - computes causal attention for its 512 queries over all 16 heads
    (causality via per-core host-built additive -inf masks pre-loaded into
     the scores PSUM by an identity matmul; the softmax denominator comes
     free from a [V_h | ones] stationary operand)
  - applies W_O and writes out^T [D, 512] for its tokens.

dtypes: QK routing in bf16 (errors only perturb tiny attention scores);
V path and W_O entirely in fp32r (full matmul rate, ~1.6e-4 error on HW).
"""

import os
import numpy as np
import ml_dtypes

import concourse.mybir as mybir
import concourse.tile as tile
from concourse import bacc
from concourse.bass_utils import run_bass_kernel_spmd

B, S, D = 4, 1024, 1024
H = 16
K_SEL = 32
N_POOL = 4096
N_CORES = 8
TOK = 512           # tokens per core
DH = D // H         # 64
PC = N_POOL // 128  # 32 pool chunks
DC = D // 128       # 8 feature chunks
TT = TOK // 128     # 4 token tiles
ST = S // 128       # 8 key tiles

BF16 = mybir.dt.bfloat16
F32 = mybir.dt.float32
F32R = mybir.dt.float32r

REPLICA_GROUPS = [[0, 1], [2, 3], [4, 5], [6, 7]]

_CACHE = {}


def _build_nc():
    nc = bacc.Bacc("TRN2", target_bir_lowering=False, debug=False,
                   num_devices=N_CORES)

    # ---- per-core external inputs -------------------------------------
    XT = nc.dram_tensor("XT", [D, TOK], BF16, kind="ExternalInput")
    XTF = nc.dram_tensor("XTF", [D, TOK], F32R, kind="ExternalInput")
    NTQKB = nc.dram_tensor("NTQKB", [PC, 128, D], BF16, kind="ExternalInput")
    NTVB = nc.dram_tensor("NTVB", [PC, 128, D], F32R, kind="ExternalInput")
    NQK = nc.dram_tensor("NQK", [N_POOL, D], BF16, kind="ExternalInput")
    NV = nc.dram_tensor("NV", [N_POOL, D], F32R, kind="ExternalInput")
    GQKT = nc.dram_tensor("GQKT", [N_POOL, 2 * TOK], BF16, kind="ExternalInput")
    GVT = nc.dram_tensor("GVT", [N_POOL, TOK], F32, kind="ExternalInput")
    MASKS = nc.dram_tensor("MASKS", [128, ST * TOK], F32R, kind="ExternalInput")
    ONES = nc.dram_tensor("ONES", [128, 128], F32R, kind="ExternalInput")
    IDN = nc.dram_tensor("IDN", [128, 128], F32R, kind="ExternalInput")
    WO = nc.dram_tensor("WO", [D, D], F32R, kind="ExternalInput")
    OT = nc.dram_tensor("OT", [D, TOK], F32, kind="ExternalOutput")

    # ---- collective staging -------------------------------------------
    kt_stage = nc.dram_tensor("kt_stage", [D, TOK], BF16)
    kt_gath = nc.dram_tensor("kt_gath", [2 * D, TOK], BF16)
    v_stage = nc.dram_tensor("v_stage", [TOK, D], F32R)
    v_gath = nc.dram_tensor("v_gath", [S, D], F32R)

    with tile.TileContext(nc) as tc:
        with (
            tc.tile_pool(name="qkt", bufs=1) as p_qkt,      # Q^T, resident
            tc.tile_pool(name="atte", bufs=1) as p_ae,      # early attn loads
        ):
            # masks / ones / identity: no deps, load first
            mask_all = p_ae.tile([128, ST * TOK], F32R, name="mask_all",
                                 tag="mask_all")
            nc.sync.dma_start(out=mask_all[:], in_=MASKS[:])
            ones_sb = p_ae.tile([128, 128], F32R, name="ones", tag="ones")
            nc.sync.dma_start(out=ones_sb[:], in_=ONES[:])
            idn_sb = p_ae.tile([128, 128], F32R, name="idn", tag="idn")
            nc.sync.dma_start(out=idn_sb[:], in_=IDN[:])
            kt_att = [p_ae.tile([128, S], BF16, name=f"ktatt{u}",
                                tag=f"ktatt{u}") for u in range(DC)]

            # =========== QK pool: route + joint recon ===================
            with tc.tile_pool(name="rqk", bufs=1) as p_rqk, \
                 tc.tile_pool(name="strmqk", bufs=1) as p_sq:
                xt_sb = []
                for kc in range(DC):
                    t = p_rqk.tile([128, TOK], BF16, name=f"xt{kc}",
                                   tag=f"xt{kc}")
                    nc.sync.dma_start(out=t[:],
                                      in_=XT[kc * 128:(kc + 1) * 128, :])
                    xt_sb.append(t)

                rq_sb, rk_sb = [], []
                with tc.tile_pool(name="ps_rt_qk", bufs=1,
                                  space="PSUM") as ps_rt:
                    for m in range(PC):
                        ntb = p_sq.tile([128, D], BF16, name=f"ntbq{m}",
                                        tag="ntbq", bufs=4)
                        nc.sync.dma_start(out=ntb[:], in_=NTQKB[m])
                        pt = ps_rt.tile([128, TOK], F32, name=f"ptq{m}",
                                        tag="pt", bufs=3)
                        for kc in range(DC):
                            nc.tensor.matmul(
                                pt[:], ntb[:, kc * 128:(kc + 1) * 128],
                                xt_sb[kc][:],
                                start=(kc == 0), stop=(kc == DC - 1))
                        gqk = p_sq.tile([128, 2 * TOK], BF16, name=f"gqk{m}",
                                        tag="gqk", bufs=4)
                        nc.sync.dma_start(
                            out=gqk[:], in_=GQKT[m * 128:(m + 1) * 128, :])
                        rq = p_rqk.tile([128, TOK], BF16, name=f"rq{m}",
                                        tag=f"rq{m}")
                        nc.vector.tensor_mul(rq[:], pt[:], gqk[:, 0:TOK])
                        rk = p_rqk.tile([128, TOK], BF16, name=f"rk{m}",
                                        tag=f"rk{m}")
                        nc.vector.tensor_mul(rk[:], pt[:], gqk[:, TOK:2 * TOK])
                        rq_sb.append(rq)
                        rk_sb.append(rk)

                # joint Q^T/K^T recon: two half-D passes over NQK
                qt_sb = [p_qkt.tile([128, TOK], BF16, name=f"qt{dt}",
                                    tag=f"qt{dt}") for dt in range(DC)]
                kt_sb = [p_rqk.tile([128, TOK], BF16, name=f"kt{dt}",
                                    tag=f"kt{dt}") for dt in range(DC)]
                with tc.tile_pool(name="ps_acc_qk", bufs=1,
                                  space="PSUM") as ps_acc:
                    for half in range(2):
                        acc_q = [ps_acc.tile([128, TOK], F32,
                                             name=f"aq{half}_{j}",
                                             tag=f"aq{j}") for j in range(4)]
                        acc_k = [ps_acc.tile([128, TOK], F32,
                                             name=f"ak{half}_{j}",
                                             tag=f"ak{j}") for j in range(4)]
                        for pc in range(PC):
                            nq = p_sq.tile([128, 512], BF16,
                                           name=f"nq{half}_{pc}",
                                           tag="nqh", bufs=4)
                            nc.sync.dma_start(
                                out=nq[:],
                                in_=NQK[pc * 128:(pc + 1) * 128,
                                        half * 512:(half + 1) * 512])
                            for j in range(4):
                                nc.tensor.matmul(
                                    acc_q[j][:], nq[:, j * 128:(j + 1) * 128],
                                    rq_sb[pc][:],
                                    start=(pc == 0), stop=(pc == PC - 1))
                            for j in range(4):
                                nc.tensor.matmul(
                                    acc_k[j][:], nq[:, j * 128:(j + 1) * 128],
                                    rk_sb[pc][:],
                                    start=(pc == 0), stop=(pc == PC - 1))
                        for j in range(4):
                            dt = half * 4 + j
                            nc.scalar.copy(qt_sb[dt][:], acc_q[j][:])
                            nc.scalar.copy(kt_sb[dt][:], acc_k[j][:])
                for dt in range(DC):
                    nc.sync.dma_start(
                        out=kt_stage[dt * 128:(dt + 1) * 128, :],
                        in_=kt_sb[dt][:])
                nc.gpsimd.collective_compute(
                    "AllGather", mybir.AluOpType.bypass,
                    replica_groups=REPLICA_GROUPS,
                    ins=[kt_stage[:]], outs=[kt_gath[:]],
                )
            # K^T for all S keys (early, before the V phase queues DMAs)
            for u in range(DC):
                for g in range(2):
                    nc.sync.dma_start(
                        out=kt_att[u][:, g * TOK:(g + 1) * TOK],
                        in_=kt_gath[g * D + u * 128: g * D + (u + 1) * 128, :])

            # =========== V pool: route + recon + exchange ===============
            with tc.tile_pool(name="rv", bufs=1) as p_rv, \
                 tc.tile_pool(name="strmv", bufs=1) as p_sv:
                xtf_sb = []
                for kc in range(DC):
                    t = p_rv.tile([128, TOK], F32R, name=f"xtf{kc}",
                                  tag=f"xtf{kc}")
                    nc.sync.dma_start(out=t[:],
                                      in_=XTF[kc * 128:(kc + 1) * 128, :])
                    xtf_sb.append(t)
                rv_sb = []
                with tc.tile_pool(name="ps_rt_v", bufs=1,
                                  space="PSUM") as ps_rt_v:
                    for m in range(PC):
                        ntb = p_sv.tile([128, D], F32R, name=f"ntbv{m}",
                                        tag="ntbv", bufs=4)
                        nc.sync.dma_start(out=ntb[:], in_=NTVB[m])
                        pt = ps_rt_v.tile([128, TOK], F32, name=f"ptv{m}",
                                          tag="pt", bufs=3)
                        for kc in range(DC):
                            nc.tensor.matmul(
                                pt[:], ntb[:, kc * 128:(kc + 1) * 128],
                                xtf_sb[kc][:],
                                start=(kc == 0), stop=(kc == DC - 1))
                        gv = p_sv.tile([128, TOK], F32, name=f"gv{m}",
                                       tag="gv", bufs=4)
                        nc.sync.dma_start(
                            out=gv[:], in_=GVT[m * 128:(m + 1) * 128, :])
                        rv = p_rv.tile([128, TOK], F32R, name=f"rv{m}",
                                       tag=f"rv{m}")
                        nc.vector.tensor_mul(rv[:], pt[:], gv[:])
                        rv_sb.append(rv)

                with tc.tile_pool(name="ps_acc_v", bufs=1,
                                  space="PSUM") as ps_acc_v:
                    v_acc = [ps_acc_v.tile([128, 512], F32, name=f"vacc{i}",
                                           tag=f"vacc{i}")
                             for i in range(2 * TT)]
                    for pc in range(PC):
                        nvch = p_sv.tile([128, D], F32R, name=f"nvch{pc}",
                                         tag="nvchunk", bufs=4)
                        nc.sync.dma_start(
                            out=nvch[:], in_=NV[pc * 128:(pc + 1) * 128, :])
                        for t in range(TT):
                            for dh in range(2):
                                nc.tensor.matmul(
                                    v_acc[t * 2 + dh][:],
                                    rv_sb[pc][:, t * 128:(t + 1) * 128],
                                    nvch[:, dh * 512:(dh + 1) * 512],
                                    start=(pc == 0), stop=(pc == PC - 1))
                    for t in range(TT):
                        for dh in range(2):
                            o = p_rv.tile([128, 512], F32R,
                                          name=f"vsb{t}_{dh}",
                                          tag=f"vsb{t}_{dh}")
                            nc.scalar.copy(o[:], v_acc[t * 2 + dh][:])
                            nc.sync.dma_start(
                                out=v_stage[t * 128:(t + 1) * 128,
                                            dh * 512:(dh + 1) * 512],
                                in_=o[:])
                nc.gpsimd.collective_compute(
                    "AllGather", mybir.AluOpType.bypass,
                    replica_groups=REPLICA_GROUPS,
                    ins=[v_stage[:]], outs=[v_gath[:]],
                )

            # ================= attention + W_O ==========================
            with tc.tile_pool(name="att", bufs=1) as p_att, \
                 tc.tile_pool(name="attw", bufs=1) as p_attw:
                # W_O resident (loads overlap V recon tail)
                wo_sb = []
                for dc in range(DC):
                    t = p_att.tile([128, D], F32R, name=f"wo{dc}",
                                   tag=f"wo{dc}")
                    nc.sync.dma_start(out=t[:],
                                      in_=WO[dc * 128:(dc + 1) * 128, :])
                    wo_sb.append(t)
                # V with interleaved [V_h | 1] layout: voall[i][:, hg*65:+65]
                vo_all = []
                for i in range(ST):
                    t = p_att.tile([128, D], F32R, name=f"vatt{i}",
                                   tag="vatt", bufs=3)
                    nc.sync.dma_start(out=t[:],
                                      in_=v_gath[i * 128:(i + 1) * 128, :])
                    va = p_att.tile([128, H * 65], F32R, name=f"voall{i}",
                                    tag=f"voall{i}")
                    dst = va[:].rearrange("p (h c) -> p h c", c=65)
                    src = t[:].rearrange("p (h c) -> p h c", c=64)
                    nc.vector.tensor_copy(dst[:, :, 0:64], src[:])
                    nc.vector.tensor_copy(
                        dst[:, :, 64:65],
                        ones_sb[:, 0:H].rearrange("p (h c) -> p h c", c=1))
                    vo_all.append(va)

                attn_sb = [p_att.tile([128, TOK], F32R, name=f"attn{u}",
                                      tag=f"attn{u}") for u in range(DC)]

                with tc.tile_pool(name="ps_att", bufs=1,
                                  space="PSUM") as ps_att:
                    for u in range(DC):
                        a_t = {}
                        for ip in range(ST // 2):
                            ps2 = {}
                            for par in range(2):
                                ps2[par] = ps_att.tile(
                                    [128, 2 * TOK], F32,
                                    name=f"pss_{u}_{par}_{ip}",
                                    tag="ps_s2", bufs=2)
                            # causal-mask preload (K=128, serial)
                            for par in range(2):
                                for hh in range(2):
                                    i = 2 * ip + hh
                                    nc.tensor.matmul(
                                        ps2[par][:, hh * TOK:(hh + 1) * TOK],
                                        idn_sb[:],
                                        mask_all[:, i * TOK:(i + 1) * TOK],
                                        start=True, stop=False,
                                        skip_group_check=True)
                            # scores: par 0/1 on disjoint row groups overlap
                            for hh in range(2):
                                i = 2 * ip + hh
                                for par in range(2):
                                    p0 = 64 * par
                                    nc.tensor.matmul(
                                        ps2[par][:, hh * TOK:(hh + 1) * TOK],
                                        kt_att[u][p0:p0 + 64,
                                                  i * 128:(i + 1) * 128],
                                        qt_sb[u][p0:p0 + 64, :],
                                        start=False, stop=True,
                                        skip_group_check=True)
                            for par in range(2):
                                a = p_attw.tile([128, 2 * TOK], F32R,
                                                name=f"a_{u}_{par}_{ip}",
                                                tag="asb", bufs=10)
                                nc.scalar.activation(
                                    a[:], ps2[par][:],
                                    mybir.ActivationFunctionType.Exp,
                                    scale=float(1.0 / np.sqrt(DH)))
                                a_t[(par, ip)] = a
                        for par in range(2):
                            hg = 2 * u + par
                            p0 = 64 * par
                            ps_o = ps_att.tile([65, TOK], F32,
                                               name=f"pso_{hg}",
                                               tag="ps_o", bufs=2)
                            for i in range(ST):
                                nc.tensor.matmul(
                                    ps_o[:],
                                    vo_all[i][:, hg * 65:(hg + 1) * 65],
                                    a_t[(par, i // 2)][
                                        :, (i % 2) * TOK:(i % 2 + 1) * TOK],
                                    start=(i == 0), stop=(i == ST - 1))
                            # denominator broadcast + normalize
                            lsb = p_attw.tile([128, TOK], F32R,
                                              name=f"lsb{hg}",
                                              tag="lsb", bufs=2)
                            with nc.allow_low_precision(
                                    reason="f32r is bit-identical to f32"):
                                nc.vector.tensor_copy(lsb[64:65, :],
                                                      ps_o[64:65, :])
                            ps_b = ps_att.tile([128, TOK], F32,
                                               name=f"psb_{hg}",
                                               tag="ps_b", bufs=2)
                            nc.tensor.matmul(
                                ps_b[:], ones_sb[64:65, :], lsb[64:65, :],
                                start=True, stop=True)
                            binv = p_attw.tile([128, TOK], F32R,
                                               name=f"binv{hg}",
                                               tag="binv", bufs=2)
                            with nc.allow_low_precision(
                                    reason="f32r is bit-identical to f32"):
                                nc.vector.reciprocal(binv[:], ps_b[:])
                            if p0 == 0:
                                nc.vector.tensor_mul(
                                    attn_sb[u][0:64, :], ps_o[0:64, :],
                                    binv[0:64, :])
                            else:
                                tmp = p_attw.tile([64, TOK], F32R,
                                                  name=f"atmp{hg}",
                                                  tag="atmp", bufs=2)
                                nc.vector.tensor_mul(tmp[:], ps_o[0:64, :],
                                                     binv[0:64, :])
                                nc.sync.dma_start(
                                    out=attn_sb[u][64:128, :], in_=tmp[:])

                # ---- W_O ----------------------------------------------
                with tc.tile_pool(name="ps_wo", bufs=1,
                                  space="PSUM") as ps_wo:
                    for dt in range(DC):
                        ps = ps_wo.tile([128, TOK], F32, name=f"psot{dt}",
                                        tag="ps_ot", bufs=2)
                        for dc in range(DC):
                            nc.tensor.matmul(
                                ps[:],
                                wo_sb[dc][:, dt * 128:(dt + 1) * 128],
                                attn_sb[dc][:],
                                start=(dc == 0), stop=(dc == DC - 1))
                        o = p_attw.tile([128, TOK], F32, name=f"ot{dt}",
                                        tag="otsb", bufs=3)
                        nc.scalar.copy(o[:], ps[:])
                        nc.sync.dma_start(
                            out=OT[dt * 128:(dt + 1) * 128, :], in_=o[:])

    nc.compile()
    return nc


def _build_inputs(inputs):
    x = np.asarray(inputs["x"], np.float32)
    g_Q = np.asarray(inputs["g_Q"], np.float32)
    g_K = np.asarray(inputs["g_K"], np.float32)
    g_V = np.asarray(inputs["g_V"], np.float32)
    ci_qk = np.asarray(inputs["ci_qk"])
    ci_v = np.asarray(inputs["ci_v"])
    nqk = np.asarray(inputs["qk_neurons"], np.float32)
    nv = np.asarray(inputs["v_neurons"], np.float32)
    wo = np.asarray(inputs["W_O"], np.float32)
    bf = ml_dtypes.bfloat16

    # Pool blocks for P^T: NTB[m][p, kc*128 + j] = N[m*128 + j, kc*128 + p]
    def pool_blocks(n):
        v = n.reshape(PC, 128, DC, 128)                     # [m, j, kc, p]
        return np.ascontiguousarray(
            v.transpose(0, 3, 2, 1).reshape(PC, 128, D))    # [m, p, (kc j)]

    ntqkb = pool_blocks(nqk).astype(bf)
    ntvb = pool_blocks(nv)
    nqk_bf = nqk.astype(bf)

    def gate_T(g_b, ci_b):
        # [N_POOL, TOK]: G^T[p, t] = sum_{n: ci[t,n]=p} g[t,n]
        out = np.zeros((N_POOL, TOK), np.float32)
        t_idx = np.repeat(np.arange(TOK), K_SEL)
        np.add.at(out, (ci_b.ravel(), t_idx), g_b.ravel())
        return out

    in_maps = []
    for c in range(N_CORES):
        b, h = c // 2, c % 2
        sl = slice(h * TOK, (h + 1) * TOK)
        masks = np.zeros((128, ST * TOK), np.float32)
        s_glob = h * TOK + np.arange(TOK)[None, :]
        for i in range(ST):
            t_glob = i * 128 + np.arange(128)[:, None]
            masks[:, i * TOK:(i + 1) * TOK] = np.where(
                t_glob <= s_glob, 0.0, -30.0 * np.sqrt(DH))
        gq = gate_T(g_Q[b, sl], ci_qk[b, sl]).astype(bf)
        gk = gate_T(g_K[b, sl], ci_qk[b, sl]).astype(bf)
        in_maps.append({
            "XT": np.ascontiguousarray(x[b, sl, :].T).astype(bf),
            "XTF": np.ascontiguousarray(x[b, sl, :].T),
            "NTQKB": ntqkb,
            "NTVB": ntvb,
            "NQK": nqk_bf,
            "NV": nv,
            "GQKT": np.concatenate([gq, gk], axis=1),
            "GVT": gate_T(g_V[b, sl], ci_v[b, sl]),
            "MASKS": masks,
            "ONES": np.ones((128, 128), np.float32),
            "IDN": np.eye(128, dtype=np.float32),
            "WO": wo,
        })
    return in_maps


def kernel(**inputs) -> np.ndarray:
    if "nc" not in _CACHE:
        _CACHE["nc"] = _build_nc()
    nc = _CACHE["nc"]
    in_maps = _build_inputs(inputs)

    trace = bool(int(os.environ.get("BASS_KERNEL_TRACE", "0")))
    res = run_bass_kernel_spmd(nc, in_maps, list(range(N_CORES)), trace=trace)
    if trace and res.exec_time_ns is not None:
        print(f"HW exec time: {res.exec_time_ns} ns")

    out = np.zeros((B, S, D), np.float32)
    for c in range(N_CORES):
        b, h = c // 2, c % 2
        ot = res.results[c]["OT"]  # [D, TOK]
        out[b, h * TOK:(h + 1) * TOK, :] = np.asarray(ot, np.float32).T
    return out


# revision 16
# speedup vs baseline: 1.2489x; 1.0437x over previous
"""Trainium2 Bass kernel for nn_AttentionCircuit (moe_routing).

Math (per batch b):
  P_qk = x_b @ qk_neurons.T            [S, NPOOL]   (dense "router" matmul)
  act[s,n] = P_qk[s, ci_qk[s,n]]
  Q = sum_n (act*gQ)[s,n] * qk_neurons[ci_qk[s,n]]  (ditto K with gK, V w/ v pool)
  causal MHA (H=16, dh=64) + W_O

Key identity: with G[s,p] = sum_{n: ci[s,n]=p} g[s,n] (host-built scatter of
the gates) the gathered reconstruction collapses to dense algebra:
  Q = (P ⊙ G_Q) @ N        =>   Q^T = N^T @ (P^T ⊙ G_Q^T)
so the MoE routing becomes two dense matmuls + one elementwise gate, with
P^T = N @ x^T computed directly in pool-major layout (no device transposes,
no gather/scatter instructions; duplicate indices handled by the host sum).

Sharding: 8 cores = (batch b = c//2) x (sequence half h = c%2). Each core:
  - routes its own 512 tokens (P^T, R^T = P^T ⊙ G^T, Q^T/K^T/V recon)
  - AllGathers K^T and V across its pair (same batch)
  - computes causal attention for its 512 queries over all 16 heads
    (causality via per-core host-built additive -inf masks pre-loaded into
     the scores PSUM by an identity matmul; the softmax denominator comes
     free from a [V_h | ones] stationary operand)
  - applies W_O and writes out^T [D, 512] for its tokens.

dtypes: QK routing in bf16 (errors only perturb tiny attention scores);
V path and W_O entirely in fp32r (full matmul rate, ~1.6e-4 error on HW).
"""

import os
import numpy as np
import ml_dtypes

import concourse.mybir as mybir
import concourse.tile as tile
from concourse import bacc
from concourse.bass_utils import run_bass_kernel_spmd

B, S, D = 4, 1024, 1024
H = 16
K_SEL = 32
N_POOL = 4096
N_CORES = 8
TOK = 512           # tokens per core
DH = D // H         # 64
PC = N_POOL // 128  # 32 pool chunks
DC = D // 128       # 8 feature chunks
TT = TOK // 128     # 4 token tiles
ST = S // 128       # 8 key tiles

BF16 = mybir.dt.bfloat16
F32 = mybir.dt.float32
F32R = mybir.dt.float32r

REPLICA_GROUPS = [[0, 1], [2, 3], [4, 5], [6, 7]]

_CACHE = {}


def _build_nc():
    nc = bacc.Bacc("TRN2", target_bir_lowering=False, debug=False,
                   num_devices=N_CORES)

    # ---- per-core external inputs -------------------------------------
    XT = nc.dram_tensor("XT", [D, TOK], BF16, kind="ExternalInput")
    XTF = nc.dram_tensor("XTF", [D, TOK], F32R, kind="ExternalInput")
    NTQKB = nc.dram_tensor("NTQKB", [PC, 128, D], BF16, kind="ExternalInput")
    NTVB = nc.dram_tensor("NTVB", [PC, 128, D], F32R, kind="ExternalInput")
    NQK = nc.dram_tensor("NQK", [N_POOL, D], BF16, kind="ExternalInput")
    NV = nc.dram_tensor("NV", [N_POOL, D], F32R, kind="ExternalInput")
    GQKT = nc.dram_tensor("GQKT", [N_POOL, 2 * TOK], BF16, kind="ExternalInput")
    GVT = nc.dram_tensor("GVT", [N_POOL, TOK], F32, kind="ExternalInput")
    MASKS = nc.dram_tensor("MASKS", [128, ST * TOK], BF16, kind="ExternalInput")
    ONES = nc.dram_tensor("ONES", [128, 128], F32R, kind="ExternalInput")
    IDN = nc.dram_tensor("IDN", [128, 128], BF16, kind="ExternalInput")
    WO = nc.dram_tensor("WO", [D, D], F32R, kind="ExternalInput")
    OT = nc.dram_tensor("OT", [D, TOK], F32, kind="ExternalOutput")

    # ---- collective staging -------------------------------------------
    kt_stage = nc.dram_tensor("kt_stage", [D, TOK], BF16)
    kt_gath = nc.dram_tensor("kt_gath", [2 * D, TOK], BF16)
    v_stage = nc.dram_tensor("v_stage", [TOK, D], F32R)
    v_gath = nc.dram_tensor("v_gath", [S, D], F32R)

    with tile.TileContext(nc) as tc:
        with (
            tc.tile_pool(name="qkt", bufs=1) as p_qkt,      # Q^T, resident
            tc.tile_pool(name="atte", bufs=1) as p_ae,      # early attn loads
        ):
            kt_att = [p_ae.tile([128, S], BF16, name=f"ktatt{u}",
                                tag=f"ktatt{u}") for u in range(DC)]

            # =========== QK pool: route + joint recon ===================
            with tc.tile_pool(name="rqk", bufs=1) as p_rqk, \
                 tc.tile_pool(name="strmqk", bufs=1) as p_sq:
                xt_sb = []
                for kc in range(DC):
                    t = p_rqk.tile([128, TOK], BF16, name=f"xt{kc}",
                                   tag=f"xt{kc}")
                    nc.sync.dma_start(out=t[:],
                                      in_=XT[kc * 128:(kc + 1) * 128, :])
                    xt_sb.append(t)

                rq_sb, rk_sb = [], []
                with tc.tile_pool(name="ps_rt_qk", bufs=1,
                                  space="PSUM") as ps_rt:
                    for m in range(PC):
                        ntb = p_sq.tile([128, D], BF16, name=f"ntbq{m}",
                                        tag="ntbq", bufs=4)
                        nc.sync.dma_start(out=ntb[:], in_=NTQKB[m])
                        pt = ps_rt.tile([128, TOK], F32, name=f"ptq{m}",
                                        tag="pt", bufs=3)
                        for kc in range(DC):
                            nc.tensor.matmul(
                                pt[:], ntb[:, kc * 128:(kc + 1) * 128],
                                xt_sb[kc][:],
                                start=(kc == 0), stop=(kc == DC - 1))
                        gqk = p_sq.tile([128, 2 * TOK], BF16, name=f"gqk{m}",
                                        tag="gqk", bufs=4)
                        nc.sync.dma_start(
                            out=gqk[:], in_=GQKT[m * 128:(m + 1) * 128, :])
                        rq = p_rqk.tile([128, TOK], BF16, name=f"rq{m}",
                                        tag=f"rq{m}")
                        nc.vector.tensor_mul(rq[:], pt[:], gqk[:, 0:TOK])
                        rk = p_rqk.tile([128, TOK], BF16, name=f"rk{m}",
                                        tag=f"rk{m}")
                        nc.vector.tensor_mul(rk[:], pt[:], gqk[:, TOK:2 * TOK])
                        rq_sb.append(rq)
                        rk_sb.append(rk)

                # joint Q^T/K^T recon: two half-D passes over NQK
                qt_sb = [p_qkt.tile([128, TOK], BF16, name=f"qt{dt}",
                                    tag=f"qt{dt}") for dt in range(DC)]
                kt_sb = [p_rqk.tile([128, TOK], BF16, name=f"kt{dt}",
                                    tag=f"kt{dt}") for dt in range(DC)]
                with tc.tile_pool(name="ps_acc_qk", bufs=1,
                                  space="PSUM") as ps_acc:
                    for half in range(2):
                        acc_q = [ps_acc.tile([128, TOK], F32,
                                             name=f"aq{half}_{j}",
                                             tag=f"aq{j}") for j in range(4)]
                        acc_k = [ps_acc.tile([128, TOK], F32,
                                             name=f"ak{half}_{j}",
                                             tag=f"ak{j}") for j in range(4)]
                        for pc in range(PC):
                            nq = p_sq.tile([128, 512], BF16,
                                           name=f"nq{half}_{pc}",
                                           tag="nqh", bufs=4)
                            nc.sync.dma_start(
                                out=nq[:],
                                in_=NQK[pc * 128:(pc + 1) * 128,
                                        half * 512:(half + 1) * 512])
                            for j in range(4):
                                nc.tensor.matmul(
                                    acc_q[j][:], nq[:, j * 128:(j + 1) * 128],
                                    rq_sb[pc][:],
                                    start=(pc == 0), stop=(pc == PC - 1))
                            for j in range(4):
                                nc.tensor.matmul(
                                    acc_k[j][:], nq[:, j * 128:(j + 1) * 128],
                                    rk_sb[pc][:],
                                    start=(pc == 0), stop=(pc == PC - 1))
                        for j in range(4):
                            dt = half * 4 + j
                            nc.scalar.copy(qt_sb[dt][:], acc_q[j][:])
                            nc.scalar.copy(kt_sb[dt][:], acc_k[j][:])
                for dt in range(DC):
                    nc.sync.dma_start(
                        out=kt_stage[dt * 128:(dt + 1) * 128, :],
                        in_=kt_sb[dt][:])
                nc.gpsimd.collective_compute(
                    "AllGather", mybir.AluOpType.bypass,
                    replica_groups=REPLICA_GROUPS,
                    ins=[kt_stage[:]], outs=[kt_gath[:]],
                )
            # attention constants + K^T loads (early, before the V phase
            # saturates the DMA queues)
            mask_all = p_ae.tile([128, ST * TOK], BF16, name="mask_all",
                                 tag="mask_all")
            nc.sync.dma_start(out=mask_all[:], in_=MASKS[:])
            ones_sb = p_ae.tile([128, 128], F32R, name="ones", tag="ones")
            nc.sync.dma_start(out=ones_sb[:], in_=ONES[:])
            idn_sb = p_ae.tile([128, 128], BF16, name="idn", tag="idn")
            nc.sync.dma_start(out=idn_sb[:], in_=IDN[:])
            for u in range(DC):
                for g in range(2):
                    nc.sync.dma_start(
                        out=kt_att[u][:, g * TOK:(g + 1) * TOK],
                        in_=kt_gath[g * D + u * 128: g * D + (u + 1) * 128, :])

            # =========== V pool: route + recon + exchange ===============
            with tc.tile_pool(name="rv", bufs=1) as p_rv, \
                 tc.tile_pool(name="strmv", bufs=1) as p_sv:
                xtf_sb = []
                for kc in range(DC):
                    t = p_rv.tile([128, TOK], F32R, name=f"xtf{kc}",
                                  tag=f"xtf{kc}")
                    nc.sync.dma_start(out=t[:],
                                      in_=XTF[kc * 128:(kc + 1) * 128, :])
                    xtf_sb.append(t)
                rv_sb = []
                with tc.tile_pool(name="ps_rt_v", bufs=1,
                                  space="PSUM") as ps_rt_v:
                    for m in range(PC):
                        ntb = p_sv.tile([128, D], F32R, name=f"ntbv{m}",
                                        tag="ntbv", bufs=4)
                        nc.sync.dma_start(out=ntb[:], in_=NTVB[m])
                        pt = ps_rt_v.tile([128, TOK], F32, name=f"ptv{m}",
                                          tag="pt", bufs=3)
                        for kc in range(DC):
                            nc.tensor.matmul(
                                pt[:], ntb[:, kc * 128:(kc + 1) * 128],
                                xtf_sb[kc][:],
                                start=(kc == 0), stop=(kc == DC - 1))
                        gv = p_sv.tile([128, TOK], F32, name=f"gv{m}",
                                       tag="gv", bufs=4)
                        nc.sync.dma_start(
                            out=gv[:], in_=GVT[m * 128:(m + 1) * 128, :])
                        rv = p_rv.tile([128, TOK], F32R, name=f"rv{m}",
                                       tag=f"rv{m}")
                        nc.vector.tensor_mul(rv[:], pt[:], gv[:])
                        rv_sb.append(rv)

                with tc.tile_pool(name="ps_acc_v", bufs=1,
                                  space="PSUM") as ps_acc_v:
                    v_acc = [ps_acc_v.tile([128, 512], F32, name=f"vacc{i}",
                                           tag=f"vacc{i}")
                             for i in range(2 * TT)]
                    for pc in range(PC):
                        nvch = p_sv.tile([128, D], F32R, name=f"nvch{pc}",
                                         tag="nvchunk", bufs=4)
                        nc.sync.dma_start(
                            out=nvch[:], in_=NV[pc * 128:(pc + 1) * 128, :])
                        for t in range(TT):
                            for dh in range(2):
                                nc.tensor.matmul(
                                    v_acc[t * 2 + dh][:],
                                    rv_sb[pc][:, t * 128:(t + 1) * 128],
                                    nvch[:, dh * 512:(dh + 1) * 512],
                                    start=(pc == 0), stop=(pc == PC - 1))
                    for t in range(TT):
                        for dh in range(2):
                            o = p_rv.tile([128, 512], F32R,
                                          name=f"vsb{t}_{dh}",
                                          tag=f"vsb{t}_{dh}")
                            nc.scalar.copy(o[:], v_acc[t * 2 + dh][:])
                            nc.sync.dma_start(
                                out=v_stage[t * 128:(t + 1) * 128,
                                            dh * 512:(dh + 1) * 512],
                                in_=o[:])
                nc.gpsimd.collective_compute(
                    "AllGather", mybir.AluOpType.bypass,
                    replica_groups=REPLICA_GROUPS,
                    ins=[v_stage[:]], outs=[v_gath[:]],
                )

            # ================= attention + W_O ==========================
            with tc.tile_pool(name="att", bufs=1) as p_att, \
                 tc.tile_pool(name="attw", bufs=1) as p_attw:
                # W_O resident (loads overlap V recon tail)
                wo_sb = []
                for dc in range(DC):
                    t = p_att.tile([128, D], F32R, name=f"wo{dc}",
                                   tag=f"wo{dc}")
                    nc.sync.dma_start(out=t[:],
                                      in_=WO[dc * 128:(dc + 1) * 128, :])
                    wo_sb.append(t)
                # V with interleaved [V_h | 1] layout: voall[i][:, hg*65:+65]
                vo_all = []
                for i in range(ST):
                    t = p_att.tile([128, D], F32R, name=f"vatt{i}",
                                   tag="vatt", bufs=3)
                    nc.sync.dma_start(out=t[:],
                                      in_=v_gath[i * 128:(i + 1) * 128, :])
                    va = p_att.tile([128, H * 65], F32R, name=f"voall{i}",
                                    tag=f"voall{i}")
                    dst = va[:].rearrange("p (h c) -> p h c", c=65)
                    src = t[:].rearrange("p (h c) -> p h c", c=64)
                    nc.vector.tensor_copy(dst[:, :, 0:64], src[:])
                    nc.vector.tensor_copy(
                        dst[:, :, 64:65],
                        ones_sb[:, 0:H].rearrange("p (h c) -> p h c", c=1))
                    vo_all.append(va)

                attn_sb = [p_att.tile([128, TOK], F32R, name=f"attn{u}",
                                      tag=f"attn{u}") for u in range(DC)]

                with tc.tile_pool(name="ps_att", bufs=1,
                                  space="PSUM") as ps_att:
                    for u in range(DC):
                        a_t = {}
                        for ip in range(ST // 2):
                            ps2 = {}
                            for par in range(2):
                                ps2[par] = ps_att.tile(
                                    [128, 2 * TOK], F32,
                                    name=f"pss_{u}_{par}_{ip}",
                                    tag="ps_s2", bufs=2)
                            # causal-mask preload (K=128, serial)
                            for par in range(2):
                                for hh in range(2):
                                    i = 2 * ip + hh
                                    nc.tensor.matmul(
                                        ps2[par][:, hh * TOK:(hh + 1) * TOK],
                                        idn_sb[:],
                                        mask_all[:, i * TOK:(i + 1) * TOK],
                                        start=True, stop=False,
                                        skip_group_check=True)
                            # scores: par 0/1 on disjoint row groups overlap
                            for hh in range(2):
                                i = 2 * ip + hh
                                for par in range(2):
                                    p0 = 64 * par
                                    nc.tensor.matmul(
                                        ps2[par][:, hh * TOK:(hh + 1) * TOK],
                                        kt_att[u][p0:p0 + 64,
                                                  i * 128:(i + 1) * 128],
                                        qt_sb[u][p0:p0 + 64, :],
                                        start=False, stop=True,
                                        skip_group_check=True)
                            for par in range(2):
                                a = p_attw.tile([128, 2 * TOK], F32R,
                                                name=f"a_{u}_{par}_{ip}",
                                                tag="asb", bufs=10)
                                nc.scalar.activation(
                                    a[:], ps2[par][:],
                                    mybir.ActivationFunctionType.Exp,
                                    scale=float(1.0 / np.sqrt(DH)))
                                a_t[(par, ip)] = a
                        for par in range(2):
                            hg = 2 * u + par
                            p0 = 64 * par
                            ps_o = ps_att.tile([65, TOK], F32,
                                               name=f"pso_{hg}",
                                               tag="ps_o", bufs=2)
                            for i in range(ST):
                                nc.tensor.matmul(
                                    ps_o[:],
                                    vo_all[i][:, hg * 65:(hg + 1) * 65],
                                    a_t[(par, i // 2)][
                                        :, (i % 2) * TOK:(i % 2 + 1) * TOK],
                                    start=(i == 0), stop=(i == ST - 1))
                            # denominator broadcast + normalize
                            lsb = p_attw.tile([128, TOK], F32R,
                                              name=f"lsb{hg}",
                                              tag="lsb", bufs=2)
                            with nc.allow_low_precision(
                                    reason="f32r is bit-identical to f32"):
                                nc.vector.tensor_copy(lsb[64:65, :],
                                                      ps_o[64:65, :])
                            ps_b = ps_att.tile([128, TOK], F32,
                                               name=f"psb_{hg}",
                                               tag="ps_b", bufs=2)
                            nc.tensor.matmul(
                                ps_b[:], ones_sb[64:65, :], lsb[64:65, :],
                                start=True, stop=True)
                            binv = p_attw.tile([128, TOK], F32,
                                               name=f"binv{hg}",
                                               tag="binv", bufs=2)
                            nc.vector.reciprocal_approx_fast(binv[:], ps_b[:])
                            if p0 == 0:
                                nc.vector.tensor_mul(
                                    attn_sb[u][0:64, :], ps_o[0:64, :],
                                    binv[0:64, :])
                            else:
                                tmp = p_attw.tile([64, TOK], F32R,
                                                  name=f"atmp{hg}",
                                                  tag="atmp", bufs=2)
                                nc.vector.tensor_mul(tmp[:], ps_o[0:64, :],
                                                     binv[0:64, :])
                                nc.sync.dma_start(
                                    out=attn_sb[u][64:128, :], in_=tmp[:])

                # ---- W_O ----------------------------------------------
                with tc.tile_pool(name="ps_wo", bufs=1,
                                  space="PSUM") as ps_wo:
                    for dt in range(DC):
                        ps = ps_wo.tile([128, TOK], F32, name=f"psot{dt}",
                                        tag="ps_ot", bufs=2)
                        for dc in range(DC):
                            nc.tensor.matmul(
                                ps[:],
                                wo_sb[dc][:, dt * 128:(dt + 1) * 128],
                                attn_sb[dc][:],
                                start=(dc == 0), stop=(dc == DC - 1))
                        o = p_attw.tile([128, TOK], F32, name=f"ot{dt}",
                                        tag="otsb", bufs=3)
                        nc.scalar.copy(o[:], ps[:])
                        nc.sync.dma_start(
                            out=OT[dt * 128:(dt + 1) * 128, :], in_=o[:])

    nc.compile()
    return nc


def _build_inputs(inputs):
    x = np.asarray(inputs["x"], np.float32)
    g_Q = np.asarray(inputs["g_Q"], np.float32)
    g_K = np.asarray(inputs["g_K"], np.float32)
    g_V = np.asarray(inputs["g_V"], np.float32)
    ci_qk = np.asarray(inputs["ci_qk"])
    ci_v = np.asarray(inputs["ci_v"])
    nqk = np.asarray(inputs["qk_neurons"], np.float32)
    nv = np.asarray(inputs["v_neurons"], np.float32)
    wo = np.asarray(inputs["W_O"], np.float32)
    bf = ml_dtypes.bfloat16

    # Pool blocks for P^T: NTB[m][p, kc*128 + j] = N[m*128 + j, kc*128 + p]
    def pool_blocks(n):
        v = n.reshape(PC, 128, DC, 128)                     # [m, j, kc, p]
        return np.ascontiguousarray(
            v.transpose(0, 3, 2, 1).reshape(PC, 128, D))    # [m, p, (kc j)]

    ntqkb = pool_blocks(nqk).astype(bf)
    ntvb = pool_blocks(nv)
    nqk_bf = nqk.astype(bf)

    def gate_T(g_b, ci_b):
        # [N_POOL, TOK]: G^T[p, t] = sum_{n: ci[t,n]=p} g[t,n]
        out = np.zeros((N_POOL, TOK), np.float32)
        t_idx = np.repeat(np.arange(TOK), K_SEL)
        np.add.at(out, (ci_b.ravel(), t_idx), g_b.ravel())
        return out

    in_maps = []
    for c in range(N_CORES):
        b, h = c // 2, c % 2
        sl = slice(h * TOK, (h + 1) * TOK)
        masks = np.zeros((128, ST * TOK), np.float32)
        s_glob = h * TOK + np.arange(TOK)[None, :]
        for i in range(ST):
            t_glob = i * 128 + np.arange(128)[:, None]
            masks[:, i * TOK:(i + 1) * TOK] = np.where(
                t_glob <= s_glob, 0.0, -30.0 * np.sqrt(DH))
        gq = gate_T(g_Q[b, sl], ci_qk[b, sl]).astype(bf)
        gk = gate_T(g_K[b, sl], ci_qk[b, sl]).astype(bf)
        in_maps.append({
            "XT": np.ascontiguousarray(x[b, sl, :].T).astype(bf),
            "XTF": np.ascontiguousarray(x[b, sl, :].T),
            "NTQKB": ntqkb,
            "NTVB": ntvb,
            "NQK": nqk_bf,
            "NV": nv,
            "GQKT": np.concatenate([gq, gk], axis=1),
            "GVT": gate_T(g_V[b, sl], ci_v[b, sl]),
            "MASKS": masks.astype(bf),
            "ONES": np.ones((128, 128), np.float32),
            "IDN": np.eye(128, dtype=np.float32).astype(bf),
            "WO": wo,
        })
    return in_maps


def kernel(**inputs) -> np.ndarray:
    if "nc" not in _CACHE:
        _CACHE["nc"] = _build_nc()
    nc = _CACHE["nc"]
    in_maps = _build_inputs(inputs)

    trace = bool(int(os.environ.get("BASS_KERNEL_TRACE", "0")))
    res = run_bass_kernel_spmd(nc, in_maps, list(range(N_CORES)), trace=trace)
    if trace and res.exec_time_ns is not None:
        print(f"HW exec time: {res.exec_time_ns} ns")

    out = np.zeros((B, S, D), np.float32)
    for c in range(N_CORES):
        b, h = c // 2, c % 2
        ot = res.results[c]["OT"]  # [D, TOK]
        out[b, h * TOK:(h + 1) * TOK, :] = np.asarray(ot, np.float32).T
    return out


# revision 17
# speedup vs baseline: 1.4496x; 1.1607x over previous
"""Trainium2 Bass kernel for nn_AttentionCircuit (moe_routing).

Math (per batch b):
  P_qk = x_b @ qk_neurons.T            [S, NPOOL]   (dense "router" matmul)
  act[s,n] = P_qk[s, ci_qk[s,n]]
  Q = sum_n (act*gQ)[s,n] * qk_neurons[ci_qk[s,n]]  (ditto K with gK, V w/ v pool)
  causal MHA (H=16, dh=64) + W_O

Key identity: with G[s,p] = sum_{n: ci[s,n]=p} g[s,n] (host-built scatter of
the gates) the gathered reconstruction collapses to dense algebra:
  Q = (P ⊙ G_Q) @ N        =>   Q^T = N^T @ (P^T ⊙ G_Q^T)
so the MoE routing becomes two dense matmuls + one elementwise gate, with
P^T = N @ x^T computed directly in pool-major layout (no device transposes,
no gather/scatter instructions; duplicate indices handled by the host sum).

Sharding: 8 cores = (batch b = c//2) x (sequence half h = c%2). Each core:
  - routes its own 512 tokens (P^T, R^T = P^T ⊙ G^T, Q^T/K^T/V recon)
  - AllGathers K^T and V across its pair (same batch)
  - computes causal attention for its 512 queries over all 16 heads
    (causality via per-core host-built additive -inf masks pre-loaded into
     the scores PSUM by an identity matmul; the softmax denominator comes
     free from a [V_h | ones] stationary operand)
  - applies W_O and writes out^T [D, 512] for its tokens.

dtypes: QK routing in bf16 (errors only perturb tiny attention scores);
V path and W_O entirely in fp32r (full matmul rate, ~1.6e-4 error on HW).
"""

import os
import numpy as np
import ml_dtypes

import concourse.mybir as mybir
import concourse.tile as tile
from concourse import bacc
from concourse.bass_utils import run_bass_kernel_spmd

B, S, D = 4, 1024, 1024
H = 16
K_SEL = 32
N_POOL = 4096
N_CORES = 8
TOK = 512           # tokens per core
DH = D // H         # 64
PC = N_POOL // 128  # 32 pool chunks
DC = D // 128       # 8 feature chunks
TT = TOK // 128     # 4 token tiles
ST = S // 128       # 8 key tiles

BF16 = mybir.dt.bfloat16
F32 = mybir.dt.float32
F32R = mybir.dt.float32r

REPLICA_GROUPS = [[0, 1], [2, 3], [4, 5], [6, 7]]

_CACHE = {}


def _build_nc():
    nc = bacc.Bacc("TRN2", target_bir_lowering=False, debug=False,
                   num_devices=N_CORES)

    # ---- per-core external inputs -------------------------------------
    XT = nc.dram_tensor("XT", [D, TOK], BF16, kind="ExternalInput")
    XTF = nc.dram_tensor("XTF", [D, TOK], F32R, kind="ExternalInput")
    NTQKB = nc.dram_tensor("NTQKB", [PC, 128, D], BF16, kind="ExternalInput")
    NTVB = nc.dram_tensor("NTVB", [PC, 128, D], F32R, kind="ExternalInput")
    NQK = nc.dram_tensor("NQK", [N_POOL, D], BF16, kind="ExternalInput")
    NV = nc.dram_tensor("NV", [N_POOL, D], F32R, kind="ExternalInput")
    GQKT = nc.dram_tensor("GQKT", [N_POOL, 2 * TOK], BF16, kind="ExternalInput")
    GVT = nc.dram_tensor("GVT", [N_POOL, TOK], F32, kind="ExternalInput")
    MASKS = nc.dram_tensor("MASKS", [128, ST * TOK], BF16, kind="ExternalInput")
    ONES = nc.dram_tensor("ONES", [128, 128], F32R, kind="ExternalInput")
    IDN = nc.dram_tensor("IDN", [128, 128], BF16, kind="ExternalInput")
    WO = nc.dram_tensor("WO", [D, D], F32R, kind="ExternalInput")
    OT = nc.dram_tensor("OT", [D, TOK], F32, kind="ExternalOutput")

    # ---- collective staging -------------------------------------------
    kt_stage = nc.dram_tensor("kt_stage", [D, TOK], BF16)
    kt_gath = nc.dram_tensor("kt_gath", [2 * D, TOK], BF16)
    v_stage = nc.dram_tensor("v_stage", [TOK, D], F32R)
    v_gath = nc.dram_tensor("v_gath", [S, D], F32R)

    with tile.TileContext(nc) as tc:
        with (
            tc.tile_pool(name="qkt", bufs=1) as p_qkt,      # Q^T, resident
            tc.tile_pool(name="atte", bufs=1) as p_ae,      # early attn loads
        ):
            kt_att = [p_ae.tile([128, S], BF16, name=f"ktatt{u}",
                                tag=f"ktatt{u}") for u in range(DC)]

            # =========== QK pool: route + joint recon ===================
            with tc.tile_pool(name="rqk", bufs=1) as p_rqk, \
                 tc.tile_pool(name="strmqk", bufs=1) as p_sq:
                xt_sb = []
                for kc in range(DC):
                    t = p_rqk.tile([128, TOK], BF16, name=f"xt{kc}",
                                   tag=f"xt{kc}")
                    nc.sync.dma_start(out=t[:],
                                      in_=XT[kc * 128:(kc + 1) * 128, :])
                    xt_sb.append(t)

                rq_sb, rk_sb = [], []
                with tc.tile_pool(name="ps_rt_qk", bufs=1,
                                  space="PSUM") as ps_rt:
                    for m in range(PC):
                        ntb = p_sq.tile([128, D], BF16, name=f"ntbq{m}",
                                        tag="ntbq", bufs=4)
                        nc.sync.dma_start(out=ntb[:], in_=NTQKB[m])
                        pt = ps_rt.tile([128, TOK], F32, name=f"ptq{m}",
                                        tag="pt", bufs=3)
                        for kc in range(DC):
                            nc.tensor.matmul(
                                pt[:], ntb[:, kc * 128:(kc + 1) * 128],
                                xt_sb[kc][:],
                                start=(kc == 0), stop=(kc == DC - 1))
                        gqk = p_sq.tile([128, 2 * TOK], BF16, name=f"gqk{m}",
                                        tag="gqk", bufs=4)
                        nc.sync.dma_start(
                            out=gqk[:], in_=GQKT[m * 128:(m + 1) * 128, :])
                        rq = p_rqk.tile([128, TOK], BF16, name=f"rq{m}",
                                        tag=f"rq{m}")
                        nc.vector.tensor_mul(rq[:], pt[:], gqk[:, 0:TOK])
                        rk = p_rqk.tile([128, TOK], BF16, name=f"rk{m}",
                                        tag=f"rk{m}")
                        nc.vector.tensor_mul(rk[:], pt[:], gqk[:, TOK:2 * TOK])
                        rq_sb.append(rq)
                        rk_sb.append(rk)

                # joint Q^T/K^T recon: two half-D passes over NQK
                qt_sb = [p_qkt.tile([128, TOK], BF16, name=f"qt{dt}",
                                    tag=f"qt{dt}") for dt in range(DC)]
                kt_sb = [p_rqk.tile([128, TOK], BF16, name=f"kt{dt}",
                                    tag=f"kt{dt}") for dt in range(DC)]
                with tc.tile_pool(name="ps_acc_qk", bufs=1,
                                  space="PSUM") as ps_acc:
                    for half in range(2):
                        acc_q = [ps_acc.tile([128, TOK], F32,
                                             name=f"aq{half}_{j}",
                                             tag=f"aq{j}") for j in range(4)]
                        acc_k = [ps_acc.tile([128, TOK], F32,
                                             name=f"ak{half}_{j}",
                                             tag=f"ak{j}") for j in range(4)]
                        for pc in range(PC):
                            nq = p_sq.tile([128, 512], BF16,
                                           name=f"nq{half}_{pc}",
                                           tag="nqh", bufs=4)
                            nc.sync.dma_start(
                                out=nq[:],
                                in_=NQK[pc * 128:(pc + 1) * 128,
                                        half * 512:(half + 1) * 512])
                            for j in range(4):
                                nc.tensor.matmul(
                                    acc_q[j][:], nq[:, j * 128:(j + 1) * 128],
                                    rq_sb[pc][:],
                                    start=(pc == 0), stop=(pc == PC - 1))
                            for j in range(4):
                                nc.tensor.matmul(
                                    acc_k[j][:], nq[:, j * 128:(j + 1) * 128],
                                    rk_sb[pc][:],
                                    start=(pc == 0), stop=(pc == PC - 1))
                        for j in range(4):
                            dt = half * 4 + j
                            nc.scalar.copy(qt_sb[dt][:], acc_q[j][:])
                            nc.scalar.copy(kt_sb[dt][:], acc_k[j][:])
                for dt in range(DC):
                    nc.sync.dma_start(
                        out=kt_stage[dt * 128:(dt + 1) * 128, :],
                        in_=kt_sb[dt][:])
                nc.gpsimd.collective_compute(
                    "AllGather", mybir.AluOpType.bypass,
                    replica_groups=REPLICA_GROUPS,
                    ins=[kt_stage[:]], outs=[kt_gath[:]],
                )
            # attention constants + K^T loads (early, before the V phase
            # saturates the DMA queues)
            mask_all = p_ae.tile([128, ST * TOK], BF16, name="mask_all",
                                 tag="mask_all")
            nc.sync.dma_start(out=mask_all[:], in_=MASKS[:])
            ones_sb = p_ae.tile([128, 128], F32R, name="ones", tag="ones")
            nc.sync.dma_start(out=ones_sb[:], in_=ONES[:])
            idn_sb = p_ae.tile([128, 128], BF16, name="idn", tag="idn")
            nc.sync.dma_start(out=idn_sb[:], in_=IDN[:])
            for u in range(DC):
                for g in range(2):
                    nc.sync.dma_start(
                        out=kt_att[u][:, g * TOK:(g + 1) * TOK],
                        in_=kt_gath[g * D + u * 128: g * D + (u + 1) * 128, :])

            # =========== V pool: route + recon + exchange ===============
            with tc.tile_pool(name="rv", bufs=1) as p_rv, \
                 tc.tile_pool(name="strmv", bufs=1) as p_sv:
                xtf_sb = []
                for kc in range(DC):
                    t = p_rv.tile([128, TOK], F32R, name=f"xtf{kc}",
                                  tag=f"xtf{kc}")
                    nc.sync.dma_start(out=t[:],
                                      in_=XTF[kc * 128:(kc + 1) * 128, :])
                    xtf_sb.append(t)
                rv_sb = []
                with tc.tile_pool(name="ps_rt_v", bufs=1,
                                  space="PSUM") as ps_rt_v:
                    for m in range(PC):
                        ntb = p_sv.tile([128, D], F32R, name=f"ntbv{m}",
                                        tag="ntbv", bufs=4)
                        nc.sync.dma_start(out=ntb[:], in_=NTVB[m])
                        pt = ps_rt_v.tile([128, TOK], F32, name=f"ptv{m}",
                                          tag="pt", bufs=3)
                        for kc in range(DC):
                            nc.tensor.matmul(
                                pt[:], ntb[:, kc * 128:(kc + 1) * 128],
                                xtf_sb[kc][:],
                                start=(kc == 0), stop=(kc == DC - 1))
                        gv = p_sv.tile([128, TOK], F32, name=f"gv{m}",
                                       tag="gv", bufs=4)
                        nc.sync.dma_start(
                            out=gv[:], in_=GVT[m * 128:(m + 1) * 128, :])
                        rv = p_rv.tile([128, TOK], F32R, name=f"rv{m}",
                                       tag=f"rv{m}")
                        nc.vector.tensor_mul(rv[:], pt[:], gv[:])
                        rv_sb.append(rv)

                with tc.tile_pool(name="ps_acc_v", bufs=1,
                                  space="PSUM") as ps_acc_v:
                    v_acc = [ps_acc_v.tile([128, 512], F32, name=f"vacc{i}",
                                           tag=f"vacc{i}")
                             for i in range(2 * TT)]
                    for pc in range(PC):
                        nvch = p_sv.tile([128, D], F32R, name=f"nvch{pc}",
                                         tag="nvchunk", bufs=4)
                        nc.sync.dma_start(
                            out=nvch[:], in_=NV[pc * 128:(pc + 1) * 128, :])
                        for t in range(TT):
                            for dh in range(2):
                                nc.tensor.matmul(
                                    v_acc[t * 2 + dh][:],
                                    rv_sb[pc][:, t * 128:(t + 1) * 128],
                                    nvch[:, dh * 512:(dh + 1) * 512],
                                    start=(pc == 0), stop=(pc == PC - 1))
                    for t in range(TT):
                        for dh in range(2):
                            o = p_rv.tile([128, 512], F32R,
                                          name=f"vsb{t}_{dh}",
                                          tag=f"vsb{t}_{dh}")
                            nc.scalar.copy(o[:], v_acc[t * 2 + dh][:])
                            nc.sync.dma_start(
                                out=v_stage[t * 128:(t + 1) * 128,
                                            dh * 512:(dh + 1) * 512],
                                in_=o[:])
                nc.gpsimd.collective_compute(
                    "AllGather", mybir.AluOpType.bypass,
                    replica_groups=REPLICA_GROUPS,
                    ins=[v_stage[:]], outs=[v_gath[:]],
                )

            # ================= attention + W_O ==========================
            with tc.tile_pool(name="att", bufs=1) as p_att, \
                 tc.tile_pool(name="attw", bufs=1) as p_attw:
                # V with interleaved [V_h | 1] layout: voall[i][:, hg*65:+65]
                vo_all = []
                for i in range(ST):
                    t = p_att.tile([128, D], F32R, name=f"vatt{i}",
                                   tag="vatt", bufs=3)
                    nc.sync.dma_start(out=t[:],
                                      in_=v_gath[i * 128:(i + 1) * 128, :])
                    va = p_att.tile([128, H * 65], F32R, name=f"voall{i}",
                                    tag=f"voall{i}")
                    dst = va[:].rearrange("p (h c) -> p h c", c=65)
                    src = t[:].rearrange("p (h c) -> p h c", c=64)
                    nc.vector.tensor_copy(dst[:, :, 0:64], src[:])
                    nc.vector.tensor_copy(
                        dst[:, :, 64:65],
                        ones_sb[:, 0:H].rearrange("p (h c) -> p h c", c=1))
                    vo_all.append(va)

                attn_sb = [p_att.tile([128, TOK], F32R, name=f"attn{u}",
                                      tag=f"attn{u}") for u in range(DC)]

                with tc.tile_pool(name="ps_att", bufs=1,
                                  space="PSUM") as ps_att:
                    def emit_scores(u):
                        a_t = {}
                        for ip in range(ST // 2):
                            ps2 = {}
                            for par in range(2):
                                ps2[par] = ps_att.tile(
                                    [128, 2 * TOK], F32,
                                    name=f"pss_{u}_{par}_{ip}",
                                    tag="ps_s2", bufs=2)
                            for par in range(2):
                                for hh in range(2):
                                    i = 2 * ip + hh
                                    nc.tensor.matmul(
                                        ps2[par][:, hh * TOK:(hh + 1) * TOK],
                                        idn_sb[:],
                                        mask_all[:, i * TOK:(i + 1) * TOK],
                                        start=True, stop=False,
                                        skip_group_check=True)
                            for hh in range(2):
                                i = 2 * ip + hh
                                for par in range(2):
                                    p0 = 64 * par
                                    nc.tensor.matmul(
                                        ps2[par][:, hh * TOK:(hh + 1) * TOK],
                                        kt_att[u][p0:p0 + 64,
                                                  i * 128:(i + 1) * 128],
                                        qt_sb[u][p0:p0 + 64, :],
                                        start=False, stop=True,
                                        skip_group_check=True)
                            for par in range(2):
                                a = p_attw.tile([128, 2 * TOK], F32R,
                                                name=f"a_{u}_{par}_{ip}",
                                                tag="asb", bufs=18)
                                nc.scalar.activation(
                                    a[:], ps2[par][:],
                                    mybir.ActivationFunctionType.Exp,
                                    scale=float(1.0 / np.sqrt(DH)))
                                a_t[(par, ip)] = a
                        return a_t

                    def emit_attnout(u, a_t):
                        for par in range(2):
                            hg = 2 * u + par
                            p0 = 64 * par
                            ps_o = ps_att.tile([65, TOK], F32,
                                               name=f"pso_{hg}",
                                               tag="ps_o", bufs=2)
                            for i in range(ST):
                                nc.tensor.matmul(
                                    ps_o[:],
                                    vo_all[i][:, hg * 65:(hg + 1) * 65],
                                    a_t[(par, i // 2)][
                                        :, (i % 2) * TOK:(i % 2 + 1) * TOK],
                                    start=(i == 0), stop=(i == ST - 1))
                            lsb = p_attw.tile([128, TOK], F32R,
                                              name=f"lsb{hg}",
                                              tag="lsb", bufs=2)
                            with nc.allow_low_precision(
                                    reason="f32r is bit-identical to f32"):
                                nc.vector.tensor_copy(lsb[64:65, :],
                                                      ps_o[64:65, :])
                            ps_b = ps_att.tile([128, TOK], F32,
                                               name=f"psb_{hg}",
                                               tag="ps_b", bufs=2)
                            nc.tensor.matmul(
                                ps_b[:], ones_sb[64:65, :], lsb[64:65, :],
                                start=True, stop=True)
                            binv = p_attw.tile([128, TOK], F32,
                                               name=f"binv{hg}",
                                               tag="binv", bufs=2)
                            nc.vector.reciprocal_approx_fast(binv[:],
                                                             ps_b[:])
                            if p0 == 0:
                                nc.vector.tensor_mul(
                                    attn_sb[u][0:64, :], ps_o[0:64, :],
                                    binv[0:64, :])
                            else:
                                tmp = p_attw.tile([64, TOK], F32R,
                                                  name=f"atmp{hg}",
                                                  tag="atmp", bufs=2)
                                nc.vector.tensor_mul(tmp[:], ps_o[0:64, :],
                                                     binv[0:64, :])
                                nc.sync.dma_start(
                                    out=attn_sb[u][64:128, :], in_=tmp[:])

                    a_prev = None
                    for u in range(DC):
                        a_cur = emit_scores(u)
                        if a_prev is not None:
                            emit_attnout(u - 1, a_prev)
                        a_prev = a_cur
                    emit_attnout(DC - 1, a_prev)

                # ---- W_O ----------------------------------------------
                with tc.tile_pool(name="ps_wo", bufs=1,
                                  space="PSUM") as ps_wo:
                    for dt in range(DC):
                        ps = ps_wo.tile([128, TOK], F32, name=f"psot{dt}",
                                        tag="ps_ot", bufs=2)
                        for dc in range(DC):
                            w = p_attw.tile([128, 128], F32R,
                                            name=f"w_{dt}_{dc}",
                                            tag="wo", bufs=6)
                            nc.sync.dma_start(
                                out=w[:],
                                in_=WO[dc * 128:(dc + 1) * 128,
                                       dt * 128:(dt + 1) * 128])
                            nc.tensor.matmul(
                                ps[:], w[:], attn_sb[dc][:],
                                start=(dc == 0), stop=(dc == DC - 1))
                        o = p_attw.tile([128, TOK], F32, name=f"ot{dt}",
                                        tag="otsb", bufs=3)
                        nc.scalar.copy(o[:], ps[:])
                        nc.sync.dma_start(
                            out=OT[dt * 128:(dt + 1) * 128, :], in_=o[:])

    nc.compile()
    return nc


def _build_inputs(inputs):
    x = np.asarray(inputs["x"], np.float32)
    g_Q = np.asarray(inputs["g_Q"], np.float32)
    g_K = np.asarray(inputs["g_K"], np.float32)
    g_V = np.asarray(inputs["g_V"], np.float32)
    ci_qk = np.asarray(inputs["ci_qk"])
    ci_v = np.asarray(inputs["ci_v"])
    nqk = np.asarray(inputs["qk_neurons"], np.float32)
    nv = np.asarray(inputs["v_neurons"], np.float32)
    wo = np.asarray(inputs["W_O"], np.float32)
    bf = ml_dtypes.bfloat16

    # Pool blocks for P^T: NTB[m][p, kc*128 + j] = N[m*128 + j, kc*128 + p]
    def pool_blocks(n):
        v = n.reshape(PC, 128, DC, 128)                     # [m, j, kc, p]
        return np.ascontiguousarray(
            v.transpose(0, 3, 2, 1).reshape(PC, 128, D))    # [m, p, (kc j)]

    ntqkb = pool_blocks(nqk).astype(bf)
    ntvb = pool_blocks(nv)
    nqk_bf = nqk.astype(bf)

    def gate_T(g_b, ci_b):
        # [N_POOL, TOK]: G^T[p, t] = sum_{n: ci[t,n]=p} g[t,n]
        out = np.zeros((N_POOL, TOK), np.float32)
        t_idx = np.repeat(np.arange(TOK), K_SEL)
        np.add.at(out, (ci_b.ravel(), t_idx), g_b.ravel())
        return out

    in_maps = []
    for c in range(N_CORES):
        b, h = c // 2, c % 2
        sl = slice(h * TOK, (h + 1) * TOK)
        masks = np.zeros((128, ST * TOK), np.float32)
        s_glob = h * TOK + np.arange(TOK)[None, :]
        for i in range(ST):
            t_glob = i * 128 + np.arange(128)[:, None]
            masks[:, i * TOK:(i + 1) * TOK] = np.where(
                t_glob <= s_glob, 0.0, -30.0 * np.sqrt(DH))
        gq = gate_T(g_Q[b, sl], ci_qk[b, sl]).astype(bf)
        gk = gate_T(g_K[b, sl], ci_qk[b, sl]).astype(bf)
        in_maps.append({
            "XT": np.ascontiguousarray(x[b, sl, :].T).astype(bf),
            "XTF": np.ascontiguousarray(x[b, sl, :].T),
            "NTQKB": ntqkb,
            "NTVB": ntvb,
            "NQK": nqk_bf,
            "NV": nv,
            "GQKT": np.concatenate([gq, gk], axis=1),
            "GVT": gate_T(g_V[b, sl], ci_v[b, sl]),
            "MASKS": masks.astype(bf),
            "ONES": np.ones((128, 128), np.float32),
            "IDN": np.eye(128, dtype=np.float32).astype(bf),
            "WO": wo,
        })
    return in_maps


def kernel(**inputs) -> np.ndarray:
    if "nc" not in _CACHE:
        _CACHE["nc"] = _build_nc()
    nc = _CACHE["nc"]
    in_maps = _build_inputs(inputs)

    trace = bool(int(os.environ.get("BASS_KERNEL_TRACE", "0")))
    res = run_bass_kernel_spmd(nc, in_maps, list(range(N_CORES)), trace=trace)
    if trace and res.exec_time_ns is not None:
        print(f"HW exec time: {res.exec_time_ns} ns")

    out = np.zeros((B, S, D), np.float32)
    for c in range(N_CORES):
        b, h = c // 2, c % 2
        ot = res.results[c]["OT"]  # [D, TOK]
        out[b, h * TOK:(h + 1) * TOK, :] = np.asarray(ot, np.float32).T
    return out
